# revision 18
# baseline (speedup 1.0000x reference)
"""Trainium2 Bass kernel for nn_BaselineGCN (2-layer GCN + BN + mean-pool + MLP head).

Strategy (8 NeuronCores):
 - Nodes sharded contiguously across cores; each core owns the in-edges of its
   node shard (dst-sharding).
 - gcn_norm factorized: table rows T[v] = dinv_v * (h @ W); per-edge weight
   w' = w_e * dinv_dst folded host-side; self-loop enters as a streamed
   diagonal tile diag(dinv_dst).
 - Per-edge gather T[src] via SWDGE dma_gather (256B rows) from an AllGather'd
   replica of the table in each core's DRAM.
 - segment_sum via TensorE: per 128-edge chunk, a host-precomputed one-hot
   B-tile (streamed from HBM, bf16) is the lhsT and PE accumulates
   B.T @ gathered_rows into the dst-block's PSUM tile. No on-device one-hot
   builds.
 - BatchNorm folded: scale S into the next layer's weights (requires S>0,
   true here), bias C' added via a rank-1 matmul into PSUM; the whole
   per-block epilogue is a single ACT Relu PSUM->SBUF drain.
 - Graph mean-pool via streamed one-hot tiles; partials AllReduce'd; the tiny
   MLP head + log_softmax run on every core.
"""
import sys
import time

sys.path.insert(0, "/opt/trn_rl_repo")

import numpy as np
import ml_dtypes

P = 128          # partitions / block size
NWIN = 4         # gather index windows (int16 range)
MAXCALL = 1024   # max indices per dma_gather (SWDGE ring capacity)
NQUEUES = 4      # SWDGE queues to round-robin
GBUFS = 12       # gather tile lookahead
BTBUFS = 6       # B-tile group lookahead (each group = 8 chunks, 256KB)
PACCB = 6        # PSUM accumulator banks
SINGLE_PACKET = True


# ---------------------------------------------------------------- host prep --

def _ceil(a, b):
    return -(-a // b)


class GCNStructure:
    """Graph partitioning + stream layout. Capacities are maxed across cores so
    the single SPMD program fits every core's data."""

    def __init__(self, src, dst, ew, batch, N, G, ncores):
        self.N, self.G, self.C = N, G, ncores
        NSH = N // ncores
        NB = _ceil(NSH, P)
        WS = _ceil(N, NWIN)
        assert N % ncores == 0
        assert WS <= 32767, "gather window exceeds int16"
        self.NSH, self.NB, self.WS = NSH, NB, WS
        self.LB = NSH - (NB - 1) * P  # rows in last block

        core = dst // NSH
        blk = (dst % NSH) // P
        # table rows reordered: newpos = [cores' first halves | cores' second halves]
        # so gather windows 0..NWIN/2-1 live entirely in table half A.
        H2 = NSH // 2
        cs, rs = src // NSH, src % NSH
        nsrc = np.where(rs < H2, cs * H2 + rs, N // 2 + cs * H2 + (rs - H2))
        self.H2 = H2
        win = nsrc // WS
        key = (core * NB + blk) * NWIN + win
        order = np.argsort(key, kind="stable")
        self.src_s, self.dst_s, self.ew_s = nsrc[order], dst[order], ew[order]
        counts = np.bincount(key, minlength=ncores * NB * NWIN).reshape(ncores, NB, NWIN)
        self.counts = counts
        cap = counts.max(axis=0)
        cap = _ceil(np.maximum(cap, 0), P) * P  # per (b, w), 0 stays 0
        self.cap = cap  # [NB, NWIN]

        # supergroups: consecutive blocks such that per-window call <= MAXCALL
        self.sgs = []
        cur = [0]
        for b in range(1, NB):
            trial = cur + [b]
            if all(cap[trial, w].sum() <= MAXCALL for w in range(NWIN)):
                cur = trial
            else:
                self.sgs.append(cur)
                cur = [b]
        self.sgs.append(cur)

        # layout: gather calls in (sg, w) order; chunks in (sg, b, w, j) order
        self.gcols = {}    # (sgi, w) -> columns in that call's tile
        self.icol = {}     # (sgi, w) -> start col (units of 16-idx) in idx stream
        self.coloff = {}   # (b, w) -> column offset inside its call tile
        sid = 0
        for sgi, sg in enumerate(self.sgs):
            for w in range(NWIN):
                cols = int(cap[sg, w].sum()) // P
                self.gcols[(sgi, w)] = cols
                self.icol[(sgi, w)] = sid
                off = 0
                for b in sg:
                    self.coloff[(b, w)] = off
                    off += int(cap[b, w]) // P
                sid += cols * 8  # n/16 = cols*128/16
        self.SID = max(sid, 8)
        self.CT = max(int(cap.sum()) // P, 1)
        self.GMAX = max(max(self.gcols.values(), default=1), 1)

        # B-tile stream: per block its chunk tiles then one self-diag tile,
        # followed (at the very end) by NB pool tiles. Groups of 8.
        self.NSTREAM = self.CT + self.NB          # message-phase tiles
        self.NTOT = self.NSTREAM + self.NB        # + pool tiles
        self.NG8 = _ceil(self.NTOT, 8)

        # per-core edge offsets into the sorted arrays, per (b, w)
        cum = np.zeros(ncores * NB * NWIN + 1, np.int64)
        np.cumsum(counts.reshape(-1), out=cum[1:])
        self.grp_start = cum  # index by (c*NB+b)*NWIN+w

        # batch / counts for pooling
        self.cnt = np.bincount(batch, minlength=G).astype(np.float32)
        self.inv_cnt = (1.0 / np.maximum(self.cnt, 1.0)).astype(np.float32)

    def core_streams(self, c, dinv, batch):
        """Build per-core device streams: idx [128, SID] i16 and the B-tile
        stream [NG8*128, 8*128] bf16 (groups of 8 chunk tiles, row-major by
        partition within group)."""
        NB, WS, NSH = self.NB, self.WS, self.NSH
        idx_cols = np.zeros((128, self.SID), np.int16)
        bt = np.zeros((self.NG8 * 8, 128, 128), np.float32)

        sh_dinv = dinv[c * NSH:(c + 1) * NSH]
        s = 0
        for sgi, sg in enumerate(self.sgs):
            # gather stream: (w, b) order
            for w in range(NWIN):
                col = self.icol[(sgi, w)]
                parts = []
                for b in sg:
                    g0 = self.grp_start[(c * NB + b) * NWIN + w]
                    g1 = self.grp_start[(c * NB + b) * NWIN + w + 1]
                    loc = (self.src_s[g0:g1] - w * WS).astype(np.int16)
                    pad = int(self.cap[b, w]) - (g1 - g0)
                    parts.append(np.concatenate([loc, np.zeros(pad, np.int16)]))
                if parts:
                    flat = np.concatenate(parts)
                    if flat.size:
                        wrapped = np.tile(flat.reshape(-1, 16).T, (8, 1))
                        idx_cols[:, col:col + flat.size // 16] = wrapped
            # B tiles: (b, w, chunk) order, then the self-diag tile per block
            for b in sg:
                for w in range(NWIN):
                    g0 = self.grp_start[(c * NB + b) * NWIN + w]
                    g1 = self.grp_start[(c * NB + b) * NWIN + w + 1]
                    n = g1 - g0
                    capbw = int(self.cap[b, w])
                    if capbw == 0:
                        continue
                    dl = np.zeros(capbw, np.int64)
                    vv = np.zeros(capbw, np.float32)
                    dloc = self.dst_s[g0:g1] - (c * NSH + b * P)
                    dl[:n] = dloc
                    # w' = w_e * dinv_dst (dst-side norm folded into the edge)
                    vv[:n] = self.ew_s[g0:g1] * sh_dinv[b * P + dloc]
                    k = capbw // P
                    for j in range(k):
                        rows = np.arange(128)
                        bt[s, rows, dl[j * P:(j + 1) * P]] = vv[j * P:(j + 1) * P]
                        s += 1
                nb = P if b < NB - 1 else self.LB
                i = np.arange(nb)
                bt[s, i, i] = sh_dinv[b * P:b * P + nb]
                s += 1
        assert s == self.NSTREAM, (s, self.NSTREAM)
        # pool tiles
        sh_batch = batch[c * NSH:(c + 1) * NSH]
        for b in range(NB):
            nb = P if b < NB - 1 else self.LB
            i = np.arange(nb)
            bt[s, i, sh_batch[b * P:b * P + nb]] = 1.0
            s += 1
        assert s == self.NTOT
        btg = bt.reshape(self.NG8, 8, 128, 128).transpose(0, 2, 1, 3)
        btg = np.ascontiguousarray(btg.reshape(self.NG8 * 128, 8 * 128)).astype(
            ml_dtypes.float8_e4m3)
        return idx_cols, btg


def _host_prep(x, edge_index, batch, edge_attr, params, ncores):
    """All index-based preprocessing + BN folding. Returns (struct, in_maps)."""
    N, INDIM = x.shape
    G = int(params["cnt_G"])
    EPS = 1e-5

    src = np.asarray(edge_index[0], np.int64)
    dst = np.asarray(edge_index[1], np.int64)
    ew = np.asarray(edge_attr, np.float32)
    batch = np.asarray(batch, np.int64)

    deg = np.bincount(dst, weights=ew.astype(np.float64), minlength=N) + 1.0
    dinv = (1.0 / np.sqrt(deg)).astype(np.float32)

    st = GCNStructure(src, dst, ew, batch, N, G, ncores)

    def bnfold(g, be, m, v, bias):
        s = (np.asarray(g) / np.sqrt(np.asarray(v) + EPS)).astype(np.float32)
        cc = ((np.asarray(bias) - np.asarray(m)) * s + np.asarray(be)).astype(np.float32)
        return s, cc

    S0, C0 = bnfold(params["g0"], params["be0"], params["m0"], params["v0"], params["b0"])
    S1, C1 = bnfold(params["g1"], params["be1"], params["m1"], params["v1"], params["b1"])
    Sf, Cf = bnfold(params["gf"], params["bef"], params["mf"], params["vf"], params["bf1"])
    assert (S0 > 0).all() and (S1 > 0).all() and (Sf > 0).all(), \
        "BN scale fold requires positive scale"

    # y_true = S * relu(acc + C/S); S folded into the consumer weights.
    W0 = np.asarray(params["W0"], np.float32)
    W1 = S0[:, None] * np.asarray(params["W1"], np.float32)
    Wf1 = S1[:, None] * np.asarray(params["Wf1"], np.float32)
    Wf2 = Sf[:, None] * np.asarray(params["Wf2"], np.float32)
    c0row = (C0 / S0).astype(np.float32)[None, :]
    c1row = (C1 / S1).astype(np.float32)[None, :]
    cfrow = (Cf / Sf).astype(np.float32)[None, :]

    HID = W0.shape[1]
    HHID = Wf1.shape[1]
    NCLASS = Wf2.shape[1]

    ident = np.eye(128, dtype=np.float32)

    NSH, NB = st.NSH, st.NB
    xv = np.asarray(x, np.float32)

    in_maps = []
    for c in range(ncores):
        idx_cols, btg = st.core_streams(c, dinv, batch)
        xpad = np.zeros((NB * P, INDIM), np.float32)
        xpad[:NSH] = xv[c * NSH:(c + 1) * NSH]
        dinvb = np.zeros((128, NB), np.float32)
        sh_dinv = dinv[c * NSH:(c + 1) * NSH]
        for b in range(NB):
            nb = P if b < NB - 1 else st.LB
            dinvb[:nb, b] = sh_dinv[b * P:b * P + nb]
        in_maps.append(dict(
            xT=np.ascontiguousarray(xpad.T),
            idxs=idx_cols, btiles=btg, dinvb=dinvb,
            w0=W0, w1=W1, wf1=Wf1, wf2=Wf2,
            c0row=c0row, c1row=c1row, cfrow=cfrow,
            ones1=np.ones((1, 128), np.float32),
            bf2b=np.tile(np.asarray(params["bf2"], np.float32)[None, :], (128, 1)),
            invcnt=st.inv_cnt[:, None].copy(),
            ident=ident,
        ))
    dims = dict(INDIM=INDIM, HID=HID, HHID=HHID, NCLASS=NCLASS)
    return st, in_maps, dims


# ------------------------------------------------------------- bass program --

def build_nc(st, dims, ncores, reps=1, fake_coll=False):
    from concourse import bass, mybir, bacc, tile

    INDIM, HID, HHID, NCLASS = dims["INDIM"], dims["HID"], dims["HHID"], dims["NCLASS"]
    N, G, NB, NSH, WS, LB = st.N, st.G, st.NB, st.NSH, st.WS, st.LB
    f32 = mybir.dt.float32
    bf16 = mybir.dt.bfloat16
    Alu = mybir.AluOpType
    Act = mybir.ActivationFunctionType

    nc = bacc.Bacc("TRN2", target_bir_lowering=False, debug=False,
                   enable_asserts=True, num_devices=ncores,
                   num_swdge_queues=NQUEUES)

    I = {}
    def inp(name, shape, dt=f32):
        I[name] = nc.dram_tensor(name, shape, dt, kind="ExternalInput")
        return I[name]

    inp("xT", [INDIM, NB * P])
    inp("idxs", [128, st.SID], mybir.dt.int16)
    f8 = mybir.dt.float8e4
    inp("btiles", [st.NG8 * 128, 8 * 128], f8)
    inp("dinvb", [128, NB])
    inp("w0", [INDIM, HID]); inp("w1", [HID, HID])
    inp("wf1", [HID, HHID]); inp("wf2", [HHID, NCLASS])
    inp("c0row", [1, HID]); inp("c1row", [1, HID]); inp("cfrow", [1, HHID])
    inp("ones1", [1, 128])
    inp("bf2b", [128, NCLASS])
    inp("invcnt", [128, 1])
    inp("ident", [128, 128])
    out_d = nc.dram_tensor("out", [G, NCLASS], f32, kind="ExternalOutput")

    qctr = [0]
    def next_q():
        q = qctr[0] % NQUEUES
        qctr[0] += 1
        return q

    with tile.TileContext(nc) as tc:
        import contextlib
        with contextlib.ExitStack() as ctx:
            const = ctx.enter_context(tc.tile_pool(name="const", bufs=1))
            stream = ctx.enter_context(tc.tile_pool(name="stream", bufs=1))
            xio = ctx.enter_context(tc.tile_pool(name="xio", bufs=3))
            xts = ctx.enter_context(tc.tile_pool(name="xts", bufs=3))
            htab = ctx.enter_context(tc.tile_pool(name="htab", bufs=2))
            ypool = ctx.enter_context(tc.tile_pool(name="ypool", bufs=NB))
            gpool = ctx.enter_context(tc.tile_pool(name="gpool", bufs=GBUFS))
            btpool = ctx.enter_context(tc.tile_pool(name="btpool", bufs=BTBUFS))
            tmp = ctx.enter_context(tc.tile_pool(name="tmp", bufs=6))
            ptrans = ctx.enter_context(tc.tile_pool(name="ptrans", bufs=1, space="PSUM"))
            phw = ctx.enter_context(tc.tile_pool(name="phw", bufs=1, space="PSUM"))
            pacc = ctx.enter_context(tc.tile_pool(name="pacc", bufs=PACCB, space="PSUM"))
            dram = ctx.enter_context(tc.tile_pool(name="dram", bufs=1, space="DRAM"))

            # ---- constants into SBUF
            C = {}
            for nm in ["w0", "w1", "wf1", "wf2", "c0row", "c1row", "cfrow",
                       "ones1", "bf2b", "invcnt", "ident"]:
                shape = list(I[nm].shape)
                tile_ = const.tile(shape, f32, tag=nm)
                nc.sync.dma_start(out=tile_[:], in_=I[nm][:])
                C[nm] = tile_
            idx_t = stream.tile([128, st.SID], mybir.dt.int16, tag="idx")
            nc.sync.dma_start(out=idx_t[:], in_=I["idxs"][:])
            dinv_t = stream.tile([128, NB], f32, tag="dnv")
            nc.sync.dma_start(out=dinv_t[:], in_=I["dinvb"][:])

            shspace = "Shared" if (ncores > 4 and reps == 1) else "Local"
            H2 = st.H2
            bnc00 = dram.tile([H2, 2 * HID], bf16, tag="bnc00")
            bnc01 = dram.tile([H2, 2 * HID], bf16, tag="bnc01")
            bnc10 = dram.tile([H2, 2 * HID], bf16, tag="bnc10")
            bnc11 = dram.tile([H2, 2 * HID], bf16, tag="bnc11")
            tab00 = dram.tile([N // 2, 2 * HID], bf16, tag="tab00", addr_space=shspace)
            tab01 = dram.tile([N // 2, 2 * HID], bf16, tag="tab01", addr_space=shspace)
            tab10 = dram.tile([N // 2, 2 * HID], bf16, tag="tab10", addr_space=shspace)
            tab11 = dram.tile([N // 2, 2 * HID], bf16, tag="tab11", addr_space=shspace)
            bounce = [[bnc00, bnc01], [bnc10, bnc11]]
            table = [[tab00, tab01], [tab10, tab11]]
            ar_in = dram.tile([G, HID], f32, tag="arin")
            ar_out = dram.tile([G, HID], f32, tag="arout", addr_space=shspace)

            # B-tile group fetch machinery (sequential stream, groups of 8)
            class BTStream:
                def __init__(self):
                    self.pos = 0
                    self.cur = None
                def reset(self, pos):
                    self.pos = pos
                    self.cur = None
                def next(self):
                    g, slot = divmod(self.pos, 8)
                    if slot == 0 or self.cur is None:
                        self.cur = btpool.tile([128, 8 * 128], f8, tag="bt")
                        nc.sync.dma_start(
                            out=self.cur[:],
                            in_=I["btiles"][g * 128:(g + 1) * 128, :])
                    self.pos += 1
                    return self.cur[:, slot * 128:(slot + 1) * 128]
            bts = BTStream()

            for _rep in range(reps):
              # ---- phase A: table0 rows = dinv * (x @ W0) -> AllGather
              htb = htab.tile([128, NB, 2 * HID], bf16, tag="htb")
              nc.vector.memset(htb[:], 0)
              xtt = None
              for b in range(NB):
                  if b % 8 == 0:
                      nbk = min(8, NB - b)
                      xtt = xio.tile([128, 8 * 128], f32, tag="xt")
                      nc.sync.dma_start(out=xtt[:, :nbk * 128],
                                        in_=I["xT"][:, b * P:(b + nbk) * P])
                  hp = phw.tile([128, HID], f32, tag="hp")
                  nc.tensor.matmul(hp[:], lhsT=xtt[:, (b % 8) * 128:(b % 8 + 1) * 128],
                                   rhs=C["w0"][:], start=True, stop=True)
                  nc.scalar.activation(out=htb[:, b, :HID], in_=hp[:], func=Act.Copy,
                                       scale=dinv_t[:, b:b + 1])
              def bounce_write(l, srctile, b, nb):
                  g0, g1 = b * P, b * P + nb
                  if g1 <= H2:
                      nc.sync.dma_start(out=bounce[l][0][g0:g1, :],
                                        in_=srctile[:nb, b, :])
                  elif g0 >= H2:
                      nc.sync.dma_start(out=bounce[l][1][g0 - H2:g1 - H2, :],
                                        in_=srctile[:nb, b, :])
                  else:
                      k = H2 - g0
                      nc.sync.dma_start(out=bounce[l][0][g0:H2, :],
                                        in_=srctile[:k, b, :])
                      nc.sync.dma_start(out=bounce[l][1][0:nb - k, :],
                                        in_=srctile[k:nb, b, :])

              for b in range(NB):
                  nb = P if b < NB - 1 else LB
                  bounce_write(0, htb, b, nb)
              h16_big = htb

              for h in range(2):
                  if fake_coll:
                      nc.sync.dma_start(out=table[0][h][0:H2, :], in_=bounce[0][h][:])
                  else:
                      nc.gpsimd.collective_compute(
                          "AllGather", Alu.bypass,
                          replica_groups=[list(range(ncores))],
                          ins=[bounce[0][h].opt()], outs=[table[0][h].opt()],
                      )

              # ---- GCN layers
              y_tiles = None
              for l in range(2):
                  crow = C["c0row"] if l == 0 else C["c1row"]
                  ydt = f32 if l == 0 else bf16
                  bts.reset(0)
                  new_tiles = []
                  for sgi, sg in enumerate(st.sgs):
                      gt = {}
                      for w in range(NWIN):
                          cols = st.gcols[(sgi, w)]
                          if cols == 0:
                              continue
                          gbf = gpool.tile([128, st.GMAX, 2 * HID], bf16, tag="g")
                          gt[w] = gbf
                          hw_, wl = divmod(w, NWIN // 2)
                          nc.gpsimd.dma_gather(
                              out_ap=gbf[:, :cols, :],
                              in_ap=table[l][hw_][wl * WS:min((wl + 1) * WS, N // 2), :],
                              idxs_ap=idx_t[:, st.icol[(sgi, w)]:st.icol[(sgi, w)] + cols * 8],
                              num_idxs=cols * P,
                              num_idxs_reg=cols * P,
                              elem_size=2 * HID,
                              queue_num=next_q(),
                              single_packet=SINGLE_PACKET,
                          )
                      for b in sg:
                          nchunks = int(st.cap[b].sum()) // P
                          total = nchunks + 2  # + self-diag + bias row
                          acc = pacc.tile([128, HID], f32, tag="acc")
                          done = 0
                          for w in range(NWIN):
                              kk = int(st.cap[b, w]) // P
                              for j in range(kk):
                                  Bt = bts.next()
                                  nc.tensor.matmul(
                                      acc[:], lhsT=Bt,
                                      rhs=gt[w][:, st.coloff[(b, w)] + j, 0:HID],
                                      start=(done == 0), stop=False)
                                  done += 1
                          Dt = bts.next()  # self-loop diag(dinv_dst)
                          nc.tensor.matmul(acc[:], lhsT=Dt,
                                           rhs=h16_big[:, b, 0:HID],
                                           start=(done == 0), stop=False)
                          nc.tensor.matmul(acc[:], lhsT=C["ones1"][:1, :],
                                           rhs=crow[:1, :], start=False, stop=True)
                          yb = ypool.tile([128, HID], ydt, tag="y")
                          nc.scalar.activation(out=yb[:], in_=acc[:], func=Act.Relu)
                          new_tiles.append(yb)
                  y_tiles = new_tiles

                  if l == 0:
                      # table1 rows = dinv * (y0 @ W1') -> AllGather
                      htb1 = htab.tile([128, NB, 2 * HID], bf16, tag="htb")
                      nc.vector.memset(htb1[:], 0)
                      for b in range(NB):
                          yb = y_tiles[b]
                          pt = ptrans.tile([128, 128], f32, tag="pt")
                          nc.tensor.transpose(pt[:HID, :], yb[:], C["ident"][:])
                          yTs = xts.tile([128, 128], f32, tag="xT")
                          nc.scalar.activation(out=yTs[:HID, :], in_=pt[:HID, :],
                                               func=Act.Copy)
                          hp = phw.tile([128, HID], f32, tag="hp")
                          nc.tensor.matmul(hp[:], lhsT=yTs[:HID, :], rhs=C["w1"][:],
                                           start=True, stop=True)
                          nc.scalar.activation(out=htb1[:, b, :HID], in_=hp[:],
                                               func=Act.Copy,
                                               scale=dinv_t[:, b:b + 1])
                      for b in range(NB):
                          nb = P if b < NB - 1 else LB
                          bounce_write(1, htb1, b, nb)
                      h16_big = htb1
                      for h in range(2):
                          if fake_coll:
                              nc.sync.dma_start(out=table[1][h][0:H2, :],
                                                in_=bounce[1][h][:])
                          else:
                              nc.gpsimd.collective_compute(
                                  "AllGather", Alu.bypass,
                                  replica_groups=[list(range(ncores))],
                                  ins=[bounce[1][h].opt()], outs=[table[1][h].opt()],
                              )

              # ---- mean pool (partial per core, AllReduce) + head
              pp = pacc.tile([128, HID], f32, tag="acc")
              for b in range(NB):
                  Bp = bts.next()
                  nc.tensor.matmul(pp[:G, :], lhsT=Bp[:, :G], rhs=y_tiles[b][:],
                                   start=(b == 0), stop=(b == NB - 1))
              pooled = tmp.tile([128, HID], f32, tag="pl")
              nc.scalar.activation(out=pooled[:G, :], in_=pp[:G, :], func=Act.Copy)
              nc.sync.dma_start(out=ar_in[:], in_=pooled[:G, :])
              if fake_coll:
                  nc.sync.dma_start(out=ar_out[:], in_=ar_in[:])
              else:
                  nc.gpsimd.collective_compute(
                      "AllReduce", Alu.add,
                      replica_groups=[list(range(ncores))],
                      ins=[ar_in.opt()], outs=[ar_out.opt()],
                  )
              pooled2 = tmp.tile([128, HID], f32, tag="pl2")
              nc.sync.dma_start(out=pooled2[:G, :], in_=ar_out[:])
              nc.vector.tensor_scalar(out=pooled2[:G, :], in0=pooled2[:G, :],
                                      scalar1=C["invcnt"][:G, :], scalar2=None,
                                      op0=Alu.mult)

              # z = relu((pooled @ Wf1') + Cf')
              pt = ptrans.tile([128, 128], f32, tag="pt")
              nc.tensor.transpose(pt[:HID, :G], pooled2[:G, :], C["ident"][:])
              pTs = xts.tile([128, 128], f32, tag="xT")
              nc.scalar.activation(out=pTs[:HID, :G], in_=pt[:HID, :G], func=Act.Copy)
              zp = phw.tile([128, HHID], f32, tag="hp")
              nc.tensor.matmul(zp[:G, :], lhsT=pTs[:HID, :G], rhs=C["wf1"][:],
                               start=True, stop=False)
              nc.tensor.matmul(zp[:G, :], lhsT=C["ones1"][:1, :G],
                               rhs=C["cfrow"][:1, :], start=False, stop=True)
              z = tmp.tile([128, HHID], f32, tag="z")
              nc.scalar.activation(out=z[:G, :], in_=zp[:G, :], func=Act.Relu)

              # logits = z @ Wf2' + bf2; out = log_softmax(logits)
              pt2 = ptrans.tile([128, 128], f32, tag="pt")
              nc.tensor.transpose(pt2[:HHID, :G], z[:G, :], C["ident"][:])
              zTs = xts.tile([128, 128], f32, tag="xT")
              nc.scalar.activation(out=zTs[:HHID, :G], in_=pt2[:HHID, :G], func=Act.Copy)
              lp = phw.tile([128, NCLASS], f32, tag="hp")
              nc.tensor.matmul(lp[:G, :], lhsT=zTs[:HHID, :G], rhs=C["wf2"][:],
                               start=True, stop=True)
              lg = tmp.tile([128, NCLASS], f32, tag="lg")
              nc.vector.tensor_tensor(out=lg[:G, :], in0=lp[:G, :], in1=C["bf2b"][:G, :], op=Alu.add)
              mx = tmp.tile([128, 1], f32, tag="mx")
              nc.vector.reduce_max(mx[:G, :], lg[:G, :], axis=mybir.AxisListType.X)
              nc.vector.tensor_scalar(out=lg[:G, :], in0=lg[:G, :], scalar1=mx[:G, :],
                                      scalar2=None, op0=Alu.subtract)
              ex = tmp.tile([128, NCLASS], f32, tag="ex")
              nc.scalar.activation(out=ex[:G, :], in_=lg[:G, :], func=Act.Exp)
              sm = tmp.tile([128, 1], f32, tag="sm")
              nc.vector.reduce_sum(sm[:G, :], ex[:G, :], axis=mybir.AxisListType.X)
              lsm = tmp.tile([128, 1], f32, tag="ls")
              nc.scalar.activation(out=lsm[:G, :], in_=sm[:G, :], func=Act.Ln)
              nc.vector.tensor_scalar(out=lg[:G, :], in0=lg[:G, :], scalar1=lsm[:G, :],
                                      scalar2=None, op0=Alu.subtract)
              nc.sync.dma_start(out=out_d[:], in_=lg[:G, :])

    nc.compile()
    return nc


# ------------------------------------------------------------ PJRT runner --

class SpmdRunner:
    """Run the compiled 8-core Bass module via PJRT (axon), mirroring
    concourse.bass2jax.run_bass_via_pjrt but keeping the jitted callable."""

    def __init__(self, nc, n_cores):
        import jax
        from jax.sharding import Mesh, PartitionSpec
        from jax.experimental.shard_map import shard_map
        from concourse import bass2jax, mybir as _mb
        from concourse.bass2jax import _bass_exec_p, install_neuronx_cc_hook
        install_neuronx_cc_hook()
        self.jax = jax
        self.nc = nc
        self.n_cores = n_cores
        partition_name = nc.partition_id_tensor.name if nc.partition_id_tensor else None
        in_names, out_names, out_avals, zero_outs = [], [], [], []
        for alloc in nc.m.functions[0].allocations:
            if not isinstance(alloc, _mb.MemoryLocationSet):
                continue
            name = alloc.memorylocations[0].name
            if alloc.kind == "ExternalInput":
                if name != partition_name:
                    in_names.append(name)
            elif alloc.kind == "ExternalOutput":
                shape = tuple(alloc.tensor_shape)
                dtype = _mb.dt.np(alloc.dtype)
                out_names.append(name)
                out_avals.append(jax.core.ShapedArray(shape, dtype))
                zero_outs.append(np.zeros(shape, dtype))
        self.in_names, self.out_names = in_names, out_names
        self.out_avals, self.zero_outs = out_avals, zero_outs
        n_params, n_outs = len(in_names), len(out_avals)
        self.n_params = n_params
        all_in_names = in_names + out_names + ([partition_name] if partition_name else [])

        def _body(*args):
            operands = list(args)
            if partition_name is not None:
                operands.append(bass2jax.partition_id_tensor())
            return tuple(_bass_exec_p.bind(
                *operands, out_avals=tuple(out_avals), in_names=tuple(all_in_names),
                out_names=tuple(out_names), lowering_input_output_aliases=(),
                sim_require_finite=True, sim_require_nnan=True, nc=nc))

        devices = jax.devices()[:n_cores]
        assert len(devices) == n_cores
        mesh = Mesh(np.asarray(devices), ("core",))
        self._sharding = jax.sharding.NamedSharding(mesh, PartitionSpec("core"))
        in_specs = (PartitionSpec("core"),) * (n_params + n_outs)
        out_specs = (PartitionSpec("core"),) * len(out_names)
        self._fn = jax.jit(
            shard_map(_body, mesh=mesh, in_specs=in_specs,
                      out_specs=out_specs, check_rep=False),
            keep_unused=True)

    def prepare(self, in_maps):
        per_core = [[np.asarray(m[name]) for name in self.in_names] for m in in_maps]
        concat_in = [np.concatenate([per_core[c][i] for c in range(self.n_cores)], axis=0)
                     for i in range(self.n_params)]
        concat_zeros = [np.zeros((self.n_cores * z.shape[0], *z.shape[1:]), z.dtype)
                        for z in self.zero_outs]
        return concat_in + concat_zeros

    def run(self, in_maps):
        out_arrs = self._fn(*self.prepare(in_maps))
        self.jax.block_until_ready(out_arrs)
        return self._split(out_arrs)

    def _split(self, out_arrs):
        return [{name: np.asarray(out_arrs[i]).reshape(self.n_cores, *self.out_avals[i].shape)[c]
                 for i, name in enumerate(self.out_names)}
                for c in range(self.n_cores)]

    def time(self, in_maps, iters=8):
        import time as _t
        args = self.prepare(in_maps)
        dargs = [self.jax.device_put(a, self._sharding) for a in args]
        out = self._fn(*dargs)
        self.jax.block_until_ready(out)
        results = self._split(out)
        times = []
        for _ in range(iters):
            t0 = _t.perf_counter()
            o = self._fn(*dargs)
            self.jax.block_until_ready(o)
            times.append(_t.perf_counter() - t0)
        return results, times


# ------------------------------------------------------------------- driver --

_CACHE = {}


def _get_runner(st, dims, ncores):
    nc = build_nc(st, dims, ncores)
    return SpmdRunner(nc, ncores)


def kernel(**inputs):
    x = np.asarray(inputs["x"], np.float32)
    edge_index = np.asarray(inputs["edge_index"])
    batch = np.asarray(inputs["batch"])
    edge_attr = np.asarray(inputs["edge_attr"], np.float32)
    G = 128
    params = {k: np.asarray(v) for k, v in inputs.items()
              if k not in ("x", "edge_index", "batch", "edge_attr", "pos")}
    params["cnt_G"] = G
    ncores = 8

    st, in_maps, dims = _host_prep(x, edge_index, batch, edge_attr, params, ncores)

    key = ("k2", x.shape, edge_index.shape, st.SID, st.CT, st.GMAX,
           tuple(tuple(s) for s in st.sgs))
    if key not in _CACHE:
        _CACHE[key] = _get_runner(st, dims, ncores)
    runner = _CACHE[key]
    _LAST.update(st=st, dims=dims, ncores=ncores, in_maps=in_maps, runner=runner)
    results = runner.run(in_maps)
    return results[0]["out"]


_LAST = {}


def estimate_exec_ns(reps=16, iters=10):
    """Per-execution device time via wall-clock delta between a 1-rep NEFF and
    an in-NEFF `reps`-times-repeated body (cancels the axon dispatch floor).
    Median-based: the axon tunnel has heavy-tailed per-call jitter."""
    import time as _t
    import jax
    st, dims, ncores = _LAST["st"], _LAST["dims"], _LAST["ncores"]
    in_maps, r1 = _LAST["in_maps"], _LAST["runner"]
    rR = SpmdRunner(build_nc(st, dims, ncores, reps=reps), ncores)
    a1 = [jax.device_put(a, r1._sharding) for a in r1.prepare(in_maps)]
    aR = [jax.device_put(a, rR._sharding) for a in rR.prepare(in_maps)]
    jax.block_until_ready(r1._fn(*a1)); jax.block_until_ready(rR._fn(*aR))
    t1s, tRs = [], []
    for _ in range(iters):
        t0 = _t.perf_counter(); jax.block_until_ready(r1._fn(*a1)); t1s.append(_t.perf_counter() - t0)
        t0 = _t.perf_counter(); jax.block_until_ready(rR._fn(*aR)); tRs.append(_t.perf_counter() - t0)
    t1s, tRs = sorted(t1s), sorted(tRs)
    per = (tRs[len(tRs) // 2] - t1s[len(t1s) // 2]) / (reps - 1)
    return per * 1e9


# revision 19
# speedup vs baseline: 1.1567x; 1.1567x over previous
"""Trainium2 Bass kernel for nn_BaselineGCN (2-layer GCN + BN + mean-pool + MLP head).

Strategy (8 NeuronCores):
 - Nodes sharded contiguously across cores; each core owns the in-edges of its
   node shard (dst-sharding).
 - gcn_norm factorized: table rows T[v] = dinv_v * (h @ W); per-edge weight
   w' = w_e * dinv_dst folded host-side; self-loop enters as a streamed
   diagonal tile diag(dinv_dst).
 - Per-edge gather T[src] via SWDGE dma_gather (256B rows) from an AllGather'd
   replica of the table in each core's DRAM.
 - segment_sum via TensorE: per 128-edge chunk, a host-precomputed one-hot
   B-tile (streamed from HBM, bf16) is the lhsT and PE accumulates
   B.T @ gathered_rows into the dst-block's PSUM tile. No on-device one-hot
   builds.
 - BatchNorm folded: scale S into the next layer's weights (requires S>0,
   true here), bias C' added via a rank-1 matmul into PSUM; the whole
   per-block epilogue is a single ACT Relu PSUM->SBUF drain.
 - Graph mean-pool via streamed one-hot tiles; partials AllReduce'd; the tiny
   MLP head + log_softmax run on every core.
"""
import sys
import time

sys.path.insert(0, "/opt/trn_rl_repo")

import numpy as np
import ml_dtypes

P = 128          # partitions / block size
NWIN = 4         # gather index windows (int16 range)
MAXCALL = 1024   # max indices per dma_gather (SWDGE ring capacity)
NQUEUES = 4      # SWDGE queues to round-robin
GBUFS = 12       # gather tile lookahead
BTBUFS = 6       # B-tile group lookahead (each group = 8 chunks, 256KB)
PACCB = 6        # PSUM accumulator banks
SINGLE_PACKET = True


# ---------------------------------------------------------------- host prep --

def _ceil(a, b):
    return -(-a // b)


class GCNStructure:
    """Graph partitioning + stream layout. Capacities are maxed across cores so
    the single SPMD program fits every core's data."""

    def __init__(self, src, dst, ew, batch, N, G, ncores):
        self.N, self.G, self.C = N, G, ncores
        NSH = N // ncores
        NB = _ceil(NSH, P)
        WS = _ceil(N, NWIN)
        assert N % ncores == 0
        assert WS <= 32767, "gather window exceeds int16"
        self.NSH, self.NB, self.WS = NSH, NB, WS
        self.LB = NSH - (NB - 1) * P  # rows in last block

        core = dst // NSH
        blk = (dst % NSH) // P
        # table rows reordered: newpos = [cores' first halves | cores' second halves]
        # so gather windows 0..NWIN/2-1 live entirely in table half A.
        H2 = NSH // 2
        cs, rs = src // NSH, src % NSH
        nsrc = np.where(rs < H2, cs * H2 + rs, N // 2 + cs * H2 + (rs - H2))
        self.H2 = H2
        win = nsrc // WS
        key = (core * NB + blk) * NWIN + win
        order = np.argsort(key, kind="stable")
        self.src_s, self.dst_s, self.ew_s = nsrc[order], dst[order], ew[order]
        counts = np.bincount(key, minlength=ncores * NB * NWIN).reshape(ncores, NB, NWIN)
        self.counts = counts
        cap = counts.max(axis=0)
        cap = _ceil(np.maximum(cap, 0), P) * P  # per (b, w), 0 stays 0
        self.cap = cap  # [NB, NWIN]

        # supergroups: consecutive blocks such that per-window call <= MAXCALL
        self.sgs = []
        cur = [0]
        for b in range(1, NB):
            trial = cur + [b]
            if all(cap[trial, w].sum() <= MAXCALL for w in range(NWIN)):
                cur = trial
            else:
                self.sgs.append(cur)
                cur = [b]
        self.sgs.append(cur)

        # layout: gather calls in (sg, w) order; chunks in (sg, b, w, j) order
        self.gcols = {}    # (sgi, w) -> columns in that call's tile
        self.icol = {}     # (sgi, w) -> start col (units of 16-idx) in idx stream
        self.coloff = {}   # (b, w) -> column offset inside its call tile
        sid = 0
        for sgi, sg in enumerate(self.sgs):
            for w in range(NWIN):
                cols = int(cap[sg, w].sum()) // P
                self.gcols[(sgi, w)] = cols
                self.icol[(sgi, w)] = sid
                off = 0
                for b in sg:
                    self.coloff[(b, w)] = off
                    off += int(cap[b, w]) // P
                sid += cols * 8  # n/16 = cols*128/16
        self.SID = max(sid, 8)
        self.CT = max(int(cap.sum()) // P, 1)
        self.GMAX = max(max(self.gcols.values(), default=1), 1)

        # B-tile stream: per block its chunk tiles then one self-diag tile,
        # followed (at the very end) by NB pool tiles. Groups of 8.
        self.NSTREAM = self.CT + self.NB          # message-phase tiles
        self.NTOT = self.NSTREAM + self.NB        # + pool tiles
        self.NG8 = _ceil(self.NTOT, 8)

        # per-core edge offsets into the sorted arrays, per (b, w)
        cum = np.zeros(ncores * NB * NWIN + 1, np.int64)
        np.cumsum(counts.reshape(-1), out=cum[1:])
        self.grp_start = cum  # index by (c*NB+b)*NWIN+w

        # batch / counts for pooling
        self.cnt = np.bincount(batch, minlength=G).astype(np.float32)
        self.inv_cnt = (1.0 / np.maximum(self.cnt, 1.0)).astype(np.float32)

    def core_streams(self, c, dinv, batch):
        """Build per-core device streams: idx [128, SID] i16 and the B-tile
        stream [NG8*128, 8*128] bf16 (groups of 8 chunk tiles, row-major by
        partition within group)."""
        NB, WS, NSH = self.NB, self.WS, self.NSH
        idx_cols = np.zeros((128, self.SID), np.int16)
        bt = np.zeros((self.NG8 * 8, 128, 128), np.float32)

        sh_dinv = dinv[c * NSH:(c + 1) * NSH]
        s = 0
        for sgi, sg in enumerate(self.sgs):
            # gather stream: (w, b) order
            for w in range(NWIN):
                col = self.icol[(sgi, w)]
                parts = []
                for b in sg:
                    g0 = self.grp_start[(c * NB + b) * NWIN + w]
                    g1 = self.grp_start[(c * NB + b) * NWIN + w + 1]
                    loc = (self.src_s[g0:g1] - w * WS).astype(np.int16)
                    pad = int(self.cap[b, w]) - (g1 - g0)
                    parts.append(np.concatenate([loc, np.zeros(pad, np.int16)]))
                if parts:
                    flat = np.concatenate(parts)
                    if flat.size:
                        wrapped = np.tile(flat.reshape(-1, 16).T, (8, 1))
                        idx_cols[:, col:col + flat.size // 16] = wrapped
            # B tiles: (b, w, chunk) order, then the self-diag tile per block
            for b in sg:
                for w in range(NWIN):
                    g0 = self.grp_start[(c * NB + b) * NWIN + w]
                    g1 = self.grp_start[(c * NB + b) * NWIN + w + 1]
                    n = g1 - g0
                    capbw = int(self.cap[b, w])
                    if capbw == 0:
                        continue
                    dl = np.zeros(capbw, np.int64)
                    vv = np.zeros(capbw, np.float32)
                    dloc = self.dst_s[g0:g1] - (c * NSH + b * P)
                    dl[:n] = dloc
                    # w' = w_e * dinv_dst (dst-side norm folded into the edge)
                    vv[:n] = self.ew_s[g0:g1] * sh_dinv[b * P + dloc]
                    k = capbw // P
                    for j in range(k):
                        rows = np.arange(128)
                        bt[s, rows, dl[j * P:(j + 1) * P]] = vv[j * P:(j + 1) * P]
                        s += 1
                nb = P if b < NB - 1 else self.LB
                i = np.arange(nb)
                bt[s, i, i] = sh_dinv[b * P:b * P + nb]
                s += 1
        assert s == self.NSTREAM, (s, self.NSTREAM)
        # pool tiles
        sh_batch = batch[c * NSH:(c + 1) * NSH]
        for b in range(NB):
            nb = P if b < NB - 1 else self.LB
            i = np.arange(nb)
            bt[s, i, sh_batch[b * P:b * P + nb]] = 1.0
            s += 1
        assert s == self.NTOT
        btg = bt.reshape(self.NG8, 8, 128, 128).transpose(0, 2, 1, 3)
        btg = np.ascontiguousarray(btg.reshape(self.NG8 * 128, 8 * 128)).astype(
            ml_dtypes.float8_e4m3)
        return idx_cols, btg


def _host_prep(x, edge_index, batch, edge_attr, params, ncores):
    """All index-based preprocessing + BN folding. Returns (struct, in_maps)."""
    N, INDIM = x.shape
    G = int(params["cnt_G"])
    EPS = 1e-5

    src = np.asarray(edge_index[0], np.int64)
    dst = np.asarray(edge_index[1], np.int64)
    ew = np.asarray(edge_attr, np.float32)
    batch = np.asarray(batch, np.int64)

    deg = np.bincount(dst, weights=ew.astype(np.float64), minlength=N) + 1.0
    dinv = (1.0 / np.sqrt(deg)).astype(np.float32)

    st = GCNStructure(src, dst, ew, batch, N, G, ncores)

    def bnfold(g, be, m, v, bias):
        s = (np.asarray(g) / np.sqrt(np.asarray(v) + EPS)).astype(np.float32)
        cc = ((np.asarray(bias) - np.asarray(m)) * s + np.asarray(be)).astype(np.float32)
        return s, cc

    S0, C0 = bnfold(params["g0"], params["be0"], params["m0"], params["v0"], params["b0"])
    S1, C1 = bnfold(params["g1"], params["be1"], params["m1"], params["v1"], params["b1"])
    Sf, Cf = bnfold(params["gf"], params["bef"], params["mf"], params["vf"], params["bf1"])
    assert (S0 > 0).all() and (S1 > 0).all() and (Sf > 0).all(), \
        "BN scale fold requires positive scale"

    # y_true = S * relu(acc + C/S); S folded into the consumer weights.
    W0 = np.asarray(params["W0"], np.float32)
    W1 = S0[:, None] * np.asarray(params["W1"], np.float32)
    Wf1 = S1[:, None] * np.asarray(params["Wf1"], np.float32)
    Wf2 = Sf[:, None] * np.asarray(params["Wf2"], np.float32)
    c0row = (C0 / S0).astype(np.float32)[None, :]
    c1row = (C1 / S1).astype(np.float32)[None, :]
    cfrow = (Cf / Sf).astype(np.float32)[None, :]

    HID = W0.shape[1]
    HHID = Wf1.shape[1]
    NCLASS = Wf2.shape[1]

    ident = np.eye(128, dtype=np.float32)

    NSH, NB = st.NSH, st.NB
    xv = np.asarray(x, np.float32)

    in_maps = []
    for c in range(ncores):
        idx_cols, btg = st.core_streams(c, dinv, batch)
        xpad = np.zeros((NB * P, INDIM), np.float32)
        xpad[:NSH] = xv[c * NSH:(c + 1) * NSH]
        dinvb = np.zeros((128, NB), np.float32)
        sh_dinv = dinv[c * NSH:(c + 1) * NSH]
        for b in range(NB):
            nb = P if b < NB - 1 else st.LB
            dinvb[:nb, b] = sh_dinv[b * P:b * P + nb]
        in_maps.append(dict(
            xT=np.ascontiguousarray(xpad.T),
            idxs=idx_cols, btiles=btg, dinvb=dinvb,
            w0=W0, w1=W1, wf1=Wf1, wf2=Wf2,
            c0row=c0row, c1row=c1row, cfrow=cfrow,
            ones1=np.ones((1, 128), np.float32),
            bf2b=np.tile(np.asarray(params["bf2"], np.float32)[None, :], (128, 1)),
            invcnt=st.inv_cnt[:, None].copy(),
            ident=ident,
        ))
    dims = dict(INDIM=INDIM, HID=HID, HHID=HHID, NCLASS=NCLASS)
    return st, in_maps, dims


# ------------------------------------------------------------- bass program --

def build_nc(st, dims, ncores, reps=1, fake_coll=False):
    from concourse import bass, mybir, bacc, tile

    INDIM, HID, HHID, NCLASS = dims["INDIM"], dims["HID"], dims["HHID"], dims["NCLASS"]
    N, G, NB, NSH, WS, LB = st.N, st.G, st.NB, st.NSH, st.WS, st.LB
    f32 = mybir.dt.float32
    bf16 = mybir.dt.bfloat16
    Alu = mybir.AluOpType
    Act = mybir.ActivationFunctionType

    nc = bacc.Bacc("TRN2", target_bir_lowering=False, debug=False,
                   enable_asserts=True, num_devices=ncores,
                   num_swdge_queues=NQUEUES)

    I = {}
    def inp(name, shape, dt=f32):
        I[name] = nc.dram_tensor(name, shape, dt, kind="ExternalInput")
        return I[name]

    inp("xT", [INDIM, NB * P])
    inp("idxs", [128, st.SID], mybir.dt.int16)
    f8 = mybir.dt.float8e4
    inp("btiles", [st.NG8 * 128, 8 * 128], f8)
    inp("dinvb", [128, NB])
    inp("w0", [INDIM, HID]); inp("w1", [HID, HID])
    inp("wf1", [HID, HHID]); inp("wf2", [HHID, NCLASS])
    inp("c0row", [1, HID]); inp("c1row", [1, HID]); inp("cfrow", [1, HHID])
    inp("ones1", [1, 128])
    inp("bf2b", [128, NCLASS])
    inp("invcnt", [128, 1])
    inp("ident", [128, 128])
    out_d = nc.dram_tensor("out", [G, NCLASS], f32, kind="ExternalOutput")

    qctr = [0]
    def next_q():
        q = qctr[0] % NQUEUES
        qctr[0] += 1
        return q

    with tile.TileContext(nc) as tc:
        import contextlib
        with contextlib.ExitStack() as ctx:
            const = ctx.enter_context(tc.tile_pool(name="const", bufs=1))
            stream = ctx.enter_context(tc.tile_pool(name="stream", bufs=1))
            xio = ctx.enter_context(tc.tile_pool(name="xio", bufs=3))
            xts = ctx.enter_context(tc.tile_pool(name="xts", bufs=3))
            htab = ctx.enter_context(tc.tile_pool(name="htab", bufs=2))
            ypool = ctx.enter_context(tc.tile_pool(name="ypool", bufs=NB))
            gpool = ctx.enter_context(tc.tile_pool(name="gpool", bufs=GBUFS))
            btpool = ctx.enter_context(tc.tile_pool(name="btpool", bufs=BTBUFS))
            tmp = ctx.enter_context(tc.tile_pool(name="tmp", bufs=6))
            ptrans = ctx.enter_context(tc.tile_pool(name="ptrans", bufs=1, space="PSUM"))
            phw = ctx.enter_context(tc.tile_pool(name="phw", bufs=1, space="PSUM"))
            pacc = ctx.enter_context(tc.tile_pool(name="pacc", bufs=PACCB, space="PSUM"))
            dram = ctx.enter_context(tc.tile_pool(name="dram", bufs=1, space="DRAM"))

            # ---- constants into SBUF
            C = {}
            for nm in ["w0", "w1", "wf1", "wf2", "c0row", "c1row", "cfrow",
                       "ones1", "bf2b", "invcnt", "ident"]:
                shape = list(I[nm].shape)
                tile_ = const.tile(shape, f32, tag=nm)
                nc.sync.dma_start(out=tile_[:], in_=I[nm][:])
                C[nm] = tile_
            idx_t = stream.tile([128, st.SID], mybir.dt.int16, tag="idx")
            nc.sync.dma_start(out=idx_t[:], in_=I["idxs"][:])
            dinv_t = stream.tile([128, NB], f32, tag="dnv")
            nc.sync.dma_start(out=dinv_t[:], in_=I["dinvb"][:])

            shspace = "Shared" if (ncores > 4 and reps == 1) else "Local"
            H2 = st.H2
            bnc00 = dram.tile([H2, 2 * HID], bf16, tag="bnc00")
            bnc01 = dram.tile([H2, 2 * HID], bf16, tag="bnc01")
            bnc10 = dram.tile([H2, 2 * HID], bf16, tag="bnc10")
            bnc11 = dram.tile([H2, 2 * HID], bf16, tag="bnc11")
            tab00 = dram.tile([N // 2, 2 * HID], bf16, tag="tab00", addr_space=shspace)
            tab01 = dram.tile([N // 2, 2 * HID], bf16, tag="tab01", addr_space=shspace)
            tab10 = dram.tile([N // 2, 2 * HID], bf16, tag="tab10", addr_space=shspace)
            tab11 = dram.tile([N // 2, 2 * HID], bf16, tag="tab11", addr_space=shspace)
            bounce = [[bnc00, bnc01], [bnc10, bnc11]]
            table = [[tab00, tab01], [tab10, tab11]]
            ar_in = dram.tile([G, HID], f32, tag="arin")
            ar_out = dram.tile([G, HID], f32, tag="arout", addr_space=shspace)

            # B-tile group fetch machinery (sequential stream, groups of 8)
            class BTStream:
                def __init__(self):
                    self.pos = 0
                    self.cur = None
                def reset(self, pos):
                    self.pos = pos
                    self.cur = None
                def next(self):
                    g, slot = divmod(self.pos, 8)
                    if slot == 0 or self.cur is None:
                        self.cur = btpool.tile([128, 8 * 128], f8, tag="bt")
                        nc.sync.dma_start(
                            out=self.cur[:],
                            in_=I["btiles"][g * 128:(g + 1) * 128, :])
                    self.pos += 1
                    return self.cur[:, slot * 128:(slot + 1) * 128]
            bts = BTStream()

            for _rep in range(reps):
              # ---- phase A: table0 rows = dinv * (x @ W0) -> AllGather
              htb = htab.tile([128, NB, 2 * HID], bf16, tag="htb")
              nc.vector.memset(htb[:], 0)
              xtt = None
              for b in range(NB):
                  if b % 8 == 0:
                      nbk = min(8, NB - b)
                      xtt = xio.tile([128, 8 * 128], f32, tag="xt")
                      nc.sync.dma_start(out=xtt[:, :nbk * 128],
                                        in_=I["xT"][:, b * P:(b + nbk) * P])
                  hp = phw.tile([128, HID], f32, tag="hp")
                  nc.tensor.matmul(hp[:], lhsT=xtt[:, (b % 8) * 128:(b % 8 + 1) * 128],
                                   rhs=C["w0"][:], start=True, stop=True)
                  nc.scalar.activation(out=htb[:, b, :HID], in_=hp[:], func=Act.Copy,
                                       scale=dinv_t[:, b:b + 1])
              def bounce_write(l, srctile, b, nb):
                  eng = nc.sync if b % 2 == 0 else nc.scalar
                  g0, g1 = b * P, b * P + nb
                  if g1 <= H2:
                      eng.dma_start(out=bounce[l][0][g0:g1, :],
                                    in_=srctile[:nb, b, :])
                  elif g0 >= H2:
                      eng.dma_start(out=bounce[l][1][g0 - H2:g1 - H2, :],
                                    in_=srctile[:nb, b, :])
                  else:
                      k = H2 - g0
                      eng.dma_start(out=bounce[l][0][g0:H2, :],
                                    in_=srctile[:k, b, :])
                      eng.dma_start(out=bounce[l][1][0:nb - k, :],
                                    in_=srctile[k:nb, b, :])

              for b in range(NB):
                  nb = P if b < NB - 1 else LB
                  bounce_write(0, htb, b, nb)
              h16_big = htb

              for h in range(2):
                  if fake_coll:
                      nc.sync.dma_start(out=table[0][h][0:H2, :], in_=bounce[0][h][:])
                  else:
                      nc.gpsimd.collective_compute(
                          "AllGather", Alu.bypass,
                          replica_groups=[list(range(ncores))],
                          ins=[bounce[0][h].opt()], outs=[table[0][h].opt()],
                      )

              # ---- GCN layers
              y_tiles = None
              for l in range(2):
                  crow = C["c0row"] if l == 0 else C["c1row"]
                  ydt = f32 if l == 0 else bf16
                  bts.reset(0)
                  new_tiles = []
                  for sgi, sg in enumerate(st.sgs):
                      qctr[0] += 1
                      gt = {}
                      for w in range(NWIN):
                          cols = st.gcols[(sgi, w)]
                          if cols == 0:
                              continue
                          gbf = gpool.tile([128, st.GMAX, 2 * HID], bf16, tag="g")
                          gt[w] = gbf
                          hw_, wl = divmod(w, NWIN // 2)
                          nc.gpsimd.dma_gather(
                              out_ap=gbf[:, :cols, :],
                              in_ap=table[l][hw_][wl * WS:min((wl + 1) * WS, N // 2), :],
                              idxs_ap=idx_t[:, st.icol[(sgi, w)]:st.icol[(sgi, w)] + cols * 8],
                              num_idxs=cols * P,
                              num_idxs_reg=cols * P,
                              elem_size=2 * HID,
                              queue_num=next_q(),
                              single_packet=SINGLE_PACKET,
                          )
                      for b in sg:
                          nchunks = int(st.cap[b].sum()) // P
                          total = nchunks + 2  # + self-diag + bias row
                          acc = pacc.tile([128, HID], f32, tag="acc")
                          done = 0
                          for w in range(NWIN):
                              kk = int(st.cap[b, w]) // P
                              for j in range(kk):
                                  Bt = bts.next()
                                  nc.tensor.matmul(
                                      acc[:], lhsT=Bt,
                                      rhs=gt[w][:, st.coloff[(b, w)] + j, 0:HID],
                                      start=(done == 0), stop=False)
                                  done += 1
                          Dt = bts.next()  # self-loop diag(dinv_dst)
                          nc.tensor.matmul(acc[:], lhsT=Dt,
                                           rhs=h16_big[:, b, 0:HID],
                                           start=(done == 0), stop=False)
                          nc.tensor.matmul(acc[:], lhsT=C["ones1"][:1, :],
                                           rhs=crow[:1, :], start=False, stop=True)
                          yb = ypool.tile([128, HID], ydt, tag="y")
                          nc.scalar.activation(out=yb[:], in_=acc[:], func=Act.Relu)
                          new_tiles.append(yb)
                  y_tiles = new_tiles

                  if l == 0:
                      # table1 rows = dinv * (y0 @ W1') -> AllGather
                      htb1 = htab.tile([128, NB, 2 * HID], bf16, tag="htb")
                      nc.vector.memset(htb1[:], 0)
                      for b in range(NB):
                          yb = y_tiles[b]
                          pt = ptrans.tile([128, 128], f32, tag="pt")
                          nc.tensor.transpose(pt[:HID, :], yb[:], C["ident"][:])
                          yTs = xts.tile([128, 128], f32, tag="xT")
                          nc.scalar.activation(out=yTs[:HID, :], in_=pt[:HID, :],
                                               func=Act.Copy)
                          hp = phw.tile([128, HID], f32, tag="hp")
                          nc.tensor.matmul(hp[:], lhsT=yTs[:HID, :], rhs=C["w1"][:],
                                           start=True, stop=True)
                          nc.scalar.activation(out=htb1[:, b, :HID], in_=hp[:],
                                               func=Act.Copy,
                                               scale=dinv_t[:, b:b + 1])
                      for b in range(NB):
                          nb = P if b < NB - 1 else LB
                          bounce_write(1, htb1, b, nb)
                      h16_big = htb1
                      for h in range(2):
                          if fake_coll:
                              nc.sync.dma_start(out=table[1][h][0:H2, :],
                                                in_=bounce[1][h][:])
                          else:
                              nc.gpsimd.collective_compute(
                                  "AllGather", Alu.bypass,
                                  replica_groups=[list(range(ncores))],
                                  ins=[bounce[1][h].opt()], outs=[table[1][h].opt()],
                              )

              # ---- mean pool (partial per core, AllReduce) + head
              pp = pacc.tile([128, HID], f32, tag="acc")
              for b in range(NB):
                  Bp = bts.next()
                  nc.tensor.matmul(pp[:G, :], lhsT=Bp[:, :G], rhs=y_tiles[b][:],
                                   start=(b == 0), stop=(b == NB - 1))
              pooled = tmp.tile([128, HID], f32, tag="pl")
              nc.scalar.activation(out=pooled[:G, :], in_=pp[:G, :], func=Act.Copy)
              nc.sync.dma_start(out=ar_in[:], in_=pooled[:G, :])
              if fake_coll:
                  nc.sync.dma_start(out=ar_out[:], in_=ar_in[:])
              else:
                  nc.gpsimd.collective_compute(
                      "AllReduce", Alu.add,
                      replica_groups=[list(range(ncores))],
                      ins=[ar_in.opt()], outs=[ar_out.opt()],
                  )
              pooled2 = tmp.tile([128, HID], f32, tag="pl2")
              nc.sync.dma_start(out=pooled2[:G, :], in_=ar_out[:])
              nc.vector.tensor_scalar(out=pooled2[:G, :], in0=pooled2[:G, :],
                                      scalar1=C["invcnt"][:G, :], scalar2=None,
                                      op0=Alu.mult)

              # z = relu((pooled @ Wf1') + Cf')
              pt = ptrans.tile([128, 128], f32, tag="pt")
              nc.tensor.transpose(pt[:HID, :G], pooled2[:G, :], C["ident"][:])
              pTs = xts.tile([128, 128], f32, tag="xT")
              nc.scalar.activation(out=pTs[:HID, :G], in_=pt[:HID, :G], func=Act.Copy)
              zp = phw.tile([128, HHID], f32, tag="hp")
              nc.tensor.matmul(zp[:G, :], lhsT=pTs[:HID, :G], rhs=C["wf1"][:],
                               start=True, stop=False)
              nc.tensor.matmul(zp[:G, :], lhsT=C["ones1"][:1, :G],
                               rhs=C["cfrow"][:1, :], start=False, stop=True)
              z = tmp.tile([128, HHID], f32, tag="z")
              nc.scalar.activation(out=z[:G, :], in_=zp[:G, :], func=Act.Relu)

              # logits = z @ Wf2' + bf2; out = log_softmax(logits)
              pt2 = ptrans.tile([128, 128], f32, tag="pt")
              nc.tensor.transpose(pt2[:HHID, :G], z[:G, :], C["ident"][:])
              zTs = xts.tile([128, 128], f32, tag="xT")
              nc.scalar.activation(out=zTs[:HHID, :G], in_=pt2[:HHID, :G], func=Act.Copy)
              lp = phw.tile([128, NCLASS], f32, tag="hp")
              nc.tensor.matmul(lp[:G, :], lhsT=zTs[:HHID, :G], rhs=C["wf2"][:],
                               start=True, stop=True)
              lg = tmp.tile([128, NCLASS], f32, tag="lg")
              nc.vector.tensor_tensor(out=lg[:G, :], in0=lp[:G, :], in1=C["bf2b"][:G, :], op=Alu.add)
              mx = tmp.tile([128, 1], f32, tag="mx")
              nc.vector.reduce_max(mx[:G, :], lg[:G, :], axis=mybir.AxisListType.X)
              nc.vector.tensor_scalar(out=lg[:G, :], in0=lg[:G, :], scalar1=mx[:G, :],
                                      scalar2=None, op0=Alu.subtract)
              ex = tmp.tile([128, NCLASS], f32, tag="ex")
              nc.scalar.activation(out=ex[:G, :], in_=lg[:G, :], func=Act.Exp)
              sm = tmp.tile([128, 1], f32, tag="sm")
              nc.vector.reduce_sum(sm[:G, :], ex[:G, :], axis=mybir.AxisListType.X)
              lsm = tmp.tile([128, 1], f32, tag="ls")
              nc.scalar.activation(out=lsm[:G, :], in_=sm[:G, :], func=Act.Ln)
              nc.vector.tensor_scalar(out=lg[:G, :], in0=lg[:G, :], scalar1=lsm[:G, :],
                                      scalar2=None, op0=Alu.subtract)
              nc.sync.dma_start(out=out_d[:], in_=lg[:G, :])

    nc.compile()
    return nc


# ------------------------------------------------------------ PJRT runner --

class SpmdRunner:
    """Run the compiled 8-core Bass module via PJRT (axon), mirroring
    concourse.bass2jax.run_bass_via_pjrt but keeping the jitted callable."""

    def __init__(self, nc, n_cores):
        import jax
        from jax.sharding import Mesh, PartitionSpec
        from jax.experimental.shard_map import shard_map
        from concourse import bass2jax, mybir as _mb
        from concourse.bass2jax import _bass_exec_p, install_neuronx_cc_hook
        install_neuronx_cc_hook()
        self.jax = jax
        self.nc = nc
        self.n_cores = n_cores
        partition_name = nc.partition_id_tensor.name if nc.partition_id_tensor else None
        in_names, out_names, out_avals, zero_outs = [], [], [], []
        for alloc in nc.m.functions[0].allocations:
            if not isinstance(alloc, _mb.MemoryLocationSet):
                continue
            name = alloc.memorylocations[0].name
            if alloc.kind == "ExternalInput":
                if name != partition_name:
                    in_names.append(name)
            elif alloc.kind == "ExternalOutput":
                shape = tuple(alloc.tensor_shape)
                dtype = _mb.dt.np(alloc.dtype)
                out_names.append(name)
                out_avals.append(jax.core.ShapedArray(shape, dtype))
                zero_outs.append(np.zeros(shape, dtype))
        self.in_names, self.out_names = in_names, out_names
        self.out_avals, self.zero_outs = out_avals, zero_outs
        n_params, n_outs = len(in_names), len(out_avals)
        self.n_params = n_params
        all_in_names = in_names + out_names + ([partition_name] if partition_name else [])

        def _body(*args):
            operands = list(args)
            if partition_name is not None:
                operands.append(bass2jax.partition_id_tensor())
            return tuple(_bass_exec_p.bind(
                *operands, out_avals=tuple(out_avals), in_names=tuple(all_in_names),
                out_names=tuple(out_names), lowering_input_output_aliases=(),
                sim_require_finite=True, sim_require_nnan=True, nc=nc))

        devices = jax.devices()[:n_cores]
        assert len(devices) == n_cores
        mesh = Mesh(np.asarray(devices), ("core",))
        self._sharding = jax.sharding.NamedSharding(mesh, PartitionSpec("core"))
        in_specs = (PartitionSpec("core"),) * (n_params + n_outs)
        out_specs = (PartitionSpec("core"),) * len(out_names)
        self._fn = jax.jit(
            shard_map(_body, mesh=mesh, in_specs=in_specs,
                      out_specs=out_specs, check_rep=False),
            keep_unused=True)

    def prepare(self, in_maps):
        per_core = [[np.asarray(m[name]) for name in self.in_names] for m in in_maps]
        concat_in = [np.concatenate([per_core[c][i] for c in range(self.n_cores)], axis=0)
                     for i in range(self.n_params)]
        concat_zeros = [np.zeros((self.n_cores * z.shape[0], *z.shape[1:]), z.dtype)
                        for z in self.zero_outs]
        return concat_in + concat_zeros

    def run(self, in_maps):
        out_arrs = self._fn(*self.prepare(in_maps))
        self.jax.block_until_ready(out_arrs)
        return self._split(out_arrs)

    def _split(self, out_arrs):
        return [{name: np.asarray(out_arrs[i]).reshape(self.n_cores, *self.out_avals[i].shape)[c]
                 for i, name in enumerate(self.out_names)}
                for c in range(self.n_cores)]

    def time(self, in_maps, iters=8):
        import time as _t
        args = self.prepare(in_maps)
        dargs = [self.jax.device_put(a, self._sharding) for a in args]
        out = self._fn(*dargs)
        self.jax.block_until_ready(out)
        results = self._split(out)
        times = []
        for _ in range(iters):
            t0 = _t.perf_counter()
            o = self._fn(*dargs)
            self.jax.block_until_ready(o)
            times.append(_t.perf_counter() - t0)
        return results, times


# ------------------------------------------------------------------- driver --

_CACHE = {}


def _get_runner(st, dims, ncores):
    nc = build_nc(st, dims, ncores)
    return SpmdRunner(nc, ncores)


def kernel(**inputs):
    x = np.asarray(inputs["x"], np.float32)
    edge_index = np.asarray(inputs["edge_index"])
    batch = np.asarray(inputs["batch"])
    edge_attr = np.asarray(inputs["edge_attr"], np.float32)
    G = 128
    params = {k: np.asarray(v) for k, v in inputs.items()
              if k not in ("x", "edge_index", "batch", "edge_attr", "pos")}
    params["cnt_G"] = G
    ncores = 8

    st, in_maps, dims = _host_prep(x, edge_index, batch, edge_attr, params, ncores)

    key = ("k2", x.shape, edge_index.shape, st.SID, st.CT, st.GMAX,
           tuple(tuple(s) for s in st.sgs))
    if key not in _CACHE:
        _CACHE[key] = _get_runner(st, dims, ncores)
    runner = _CACHE[key]
    _LAST.update(st=st, dims=dims, ncores=ncores, in_maps=in_maps, runner=runner)
    results = runner.run(in_maps)
    return results[0]["out"]


_LAST = {}


def estimate_exec_ns(reps=16, iters=10):
    """Per-execution device time via wall-clock delta between a 1-rep NEFF and
    an in-NEFF `reps`-times-repeated body (cancels the axon dispatch floor).
    Median-based: the axon tunnel has heavy-tailed per-call jitter."""
    import time as _t
    import jax
    st, dims, ncores = _LAST["st"], _LAST["dims"], _LAST["ncores"]
    in_maps, r1 = _LAST["in_maps"], _LAST["runner"]
    rR = SpmdRunner(build_nc(st, dims, ncores, reps=reps), ncores)
    a1 = [jax.device_put(a, r1._sharding) for a in r1.prepare(in_maps)]
    aR = [jax.device_put(a, rR._sharding) for a in rR.prepare(in_maps)]
    jax.block_until_ready(r1._fn(*a1)); jax.block_until_ready(rR._fn(*aR))
    t1s, tRs = [], []
    for _ in range(iters):
        t0 = _t.perf_counter(); jax.block_until_ready(r1._fn(*a1)); t1s.append(_t.perf_counter() - t0)
        t0 = _t.perf_counter(); jax.block_until_ready(rR._fn(*aR)); tRs.append(_t.perf_counter() - t0)
    t1s, tRs = sorted(t1s), sorted(tRs)
    per = (tRs[len(tRs) // 2] - t1s[len(t1s) // 2]) / (reps - 1)
    return per * 1e9


# revision 20
# speedup vs baseline: 1.2922x; 1.1172x over previous
"""Trainium2 Bass kernel for nn_BaselineGCN (2-layer GCN + BN + mean-pool + MLP head).

Strategy (8 NeuronCores):
 - Nodes sharded contiguously across cores; each core owns the in-edges of its
   node shard (dst-sharding).
 - gcn_norm factorized: table rows T[v] = dinv_v * (h @ W); per-edge weight
   w' = w_e * dinv_dst folded host-side; self-loop enters as a streamed
   diagonal tile diag(dinv_dst).
 - Per-edge gather T[src] via SWDGE dma_gather (256B rows) from an AllGather'd
   replica of the table in each core's DRAM.
 - segment_sum via TensorE: per 128-edge chunk, a host-precomputed one-hot
   B-tile (streamed from HBM, bf16) is the lhsT and PE accumulates
   B.T @ gathered_rows into the dst-block's PSUM tile. No on-device one-hot
   builds.
 - BatchNorm folded: scale S into the next layer's weights (requires S>0,
   true here), bias C' added via a rank-1 matmul into PSUM; the whole
   per-block epilogue is a single ACT Relu PSUM->SBUF drain.
 - Graph mean-pool via streamed one-hot tiles; partials AllReduce'd; the tiny
   MLP head + log_softmax run on every core.
"""
import sys
import time

sys.path.insert(0, "/opt/trn_rl_repo")

import numpy as np
import ml_dtypes

P = 128          # partitions / block size
NWIN = 4         # gather index windows (int16 range)
MAXCALL = 1024   # max indices per dma_gather (SWDGE ring capacity)
NQUEUES = 4      # SWDGE queues to round-robin
GBUFS = 12       # gather tile lookahead
BTBUFS = 6       # B-tile group lookahead (each group = 8 chunks, 256KB)
PACCB = 5        # PSUM accumulator banks
SINGLE_PACKET = True


# ---------------------------------------------------------------- host prep --

def _ceil(a, b):
    return -(-a // b)


class GCNStructure:
    """Graph partitioning + stream layout. Capacities are maxed across cores so
    the single SPMD program fits every core's data."""

    def __init__(self, src, dst, ew, batch, N, G, ncores):
        self.N, self.G, self.C = N, G, ncores
        NSH = N // ncores
        NB = _ceil(NSH, P)
        WS = _ceil(N, NWIN)
        assert N % ncores == 0
        assert WS <= 32767, "gather window exceeds int16"
        self.NSH, self.NB, self.WS = NSH, NB, WS
        self.LB = NSH - (NB - 1) * P  # rows in last block

        core = dst // NSH
        blk = (dst % NSH) // P
        # table rows reordered: newpos = [cores' first halves | cores' second halves]
        # so gather windows 0..NWIN/2-1 live entirely in table half A.
        H2 = NSH // 2
        cs, rs = src // NSH, src % NSH
        nsrc = np.where(rs < H2, cs * H2 + rs, N // 2 + cs * H2 + (rs - H2))
        self.H2 = H2
        win = nsrc // WS
        key = (core * NB + blk) * NWIN + win
        order = np.argsort(key, kind="stable")
        self.src_s, self.dst_s, self.ew_s = nsrc[order], dst[order], ew[order]
        counts = np.bincount(key, minlength=ncores * NB * NWIN).reshape(ncores, NB, NWIN)
        self.counts = counts
        cap = counts.max(axis=0)
        cap = _ceil(np.maximum(cap, 0), P) * P  # per (b, w), 0 stays 0
        self.cap = cap  # [NB, NWIN]

        # supergroups: consecutive blocks such that per-window call <= MAXCALL
        self.sgs = []
        cur = [0]
        for b in range(1, NB):
            trial = cur + [b]
            if all(cap[trial, w].sum() <= MAXCALL for w in range(NWIN)):
                cur = trial
            else:
                self.sgs.append(cur)
                cur = [b]
        self.sgs.append(cur)

        # layout: gather calls in (sg, w) order; chunks in (sg, b, w, j) order
        self.gcols = {}    # (sgi, w) -> columns in that call's tile
        self.icol = {}     # (sgi, w) -> start col (units of 16-idx) in idx stream
        self.coloff = {}   # (b, w) -> column offset inside its call tile
        sid = 0
        for sgi, sg in enumerate(self.sgs):
            for w in range(NWIN):
                cols = int(cap[sg, w].sum()) // P
                self.gcols[(sgi, w)] = cols
                self.icol[(sgi, w)] = sid
                off = 0
                for b in sg:
                    self.coloff[(b, w)] = off
                    off += int(cap[b, w]) // P
                sid += cols * 8  # n/16 = cols*128/16
        self.SID = max(sid, 8)
        self.CT = max(int(cap.sum()) // P, 1)
        self.GMAX = max(max(self.gcols.values(), default=1), 1)

        # B-tile stream: per block its chunk tiles then one self-diag tile,
        # followed (at the very end) by NB pool tiles. Groups of 8.
        self.NSTREAM = self.CT + self.NB          # message-phase tiles
        self.NTOT = self.NSTREAM + self.NB        # + pool tiles
        self.NG8 = _ceil(self.NTOT, 8)

        # per-core edge offsets into the sorted arrays, per (b, w)
        cum = np.zeros(ncores * NB * NWIN + 1, np.int64)
        np.cumsum(counts.reshape(-1), out=cum[1:])
        self.grp_start = cum  # index by (c*NB+b)*NWIN+w

        # batch / counts for pooling
        self.cnt = np.bincount(batch, minlength=G).astype(np.float32)
        self.inv_cnt = (1.0 / np.maximum(self.cnt, 1.0)).astype(np.float32)

    def core_streams(self, c, dinv, batch):
        """Build per-core device streams: idx [128, SID] i16 and the B-tile
        stream [NG8*128, 8*128] bf16 (groups of 8 chunk tiles, row-major by
        partition within group)."""
        NB, WS, NSH = self.NB, self.WS, self.NSH
        idx_cols = np.zeros((128, self.SID), np.int16)
        bt = np.zeros((self.NG8 * 8, 128, 128), np.float32)

        sh_dinv = dinv[c * NSH:(c + 1) * NSH]
        s = 0
        for sgi, sg in enumerate(self.sgs):
            # gather stream: (w, b) order
            for w in range(NWIN):
                col = self.icol[(sgi, w)]
                parts = []
                for b in sg:
                    g0 = self.grp_start[(c * NB + b) * NWIN + w]
                    g1 = self.grp_start[(c * NB + b) * NWIN + w + 1]
                    loc = (self.src_s[g0:g1] - w * WS).astype(np.int16)
                    pad = int(self.cap[b, w]) - (g1 - g0)
                    parts.append(np.concatenate([loc, np.zeros(pad, np.int16)]))
                if parts:
                    flat = np.concatenate(parts)
                    if flat.size:
                        wrapped = np.tile(flat.reshape(-1, 16).T, (8, 1))
                        idx_cols[:, col:col + flat.size // 16] = wrapped
            # B tiles: (b, w, chunk) order, then the self-diag tile per block
            for b in sg:
                for w in range(NWIN):
                    g0 = self.grp_start[(c * NB + b) * NWIN + w]
                    g1 = self.grp_start[(c * NB + b) * NWIN + w + 1]
                    n = g1 - g0
                    capbw = int(self.cap[b, w])
                    if capbw == 0:
                        continue
                    dl = np.zeros(capbw, np.int64)
                    vv = np.zeros(capbw, np.float32)
                    dloc = self.dst_s[g0:g1] - (c * NSH + b * P)
                    dl[:n] = dloc
                    # w' = w_e * dinv_dst (dst-side norm folded into the edge)
                    vv[:n] = self.ew_s[g0:g1] * sh_dinv[b * P + dloc]
                    k = capbw // P
                    for j in range(k):
                        rows = np.arange(128)
                        bt[s, rows, dl[j * P:(j + 1) * P]] = vv[j * P:(j + 1) * P]
                        s += 1
                nb = P if b < NB - 1 else self.LB
                i = np.arange(nb)
                bt[s, i, i] = sh_dinv[b * P:b * P + nb]
                s += 1
        assert s == self.NSTREAM, (s, self.NSTREAM)
        # pool tiles
        sh_batch = batch[c * NSH:(c + 1) * NSH]
        for b in range(NB):
            nb = P if b < NB - 1 else self.LB
            i = np.arange(nb)
            bt[s, i, sh_batch[b * P:b * P + nb]] = 1.0
            s += 1
        assert s == self.NTOT
        btg = bt.reshape(self.NG8, 8, 128, 128).transpose(0, 2, 1, 3)
        btg = np.ascontiguousarray(btg.reshape(self.NG8 * 128, 8 * 128)).astype(
            ml_dtypes.float8_e4m3)
        return idx_cols, btg


def _host_prep(x, edge_index, batch, edge_attr, params, ncores):
    """All index-based preprocessing + BN folding. Returns (struct, in_maps)."""
    N, INDIM = x.shape
    G = int(params["cnt_G"])
    EPS = 1e-5

    src = np.asarray(edge_index[0], np.int64)
    dst = np.asarray(edge_index[1], np.int64)
    ew = np.asarray(edge_attr, np.float32)
    batch = np.asarray(batch, np.int64)

    deg = np.bincount(dst, weights=ew.astype(np.float64), minlength=N) + 1.0
    dinv = (1.0 / np.sqrt(deg)).astype(np.float32)

    st = GCNStructure(src, dst, ew, batch, N, G, ncores)

    def bnfold(g, be, m, v, bias):
        s = (np.asarray(g) / np.sqrt(np.asarray(v) + EPS)).astype(np.float32)
        cc = ((np.asarray(bias) - np.asarray(m)) * s + np.asarray(be)).astype(np.float32)
        return s, cc

    S0, C0 = bnfold(params["g0"], params["be0"], params["m0"], params["v0"], params["b0"])
    S1, C1 = bnfold(params["g1"], params["be1"], params["m1"], params["v1"], params["b1"])
    Sf, Cf = bnfold(params["gf"], params["bef"], params["mf"], params["vf"], params["bf1"])
    assert (S0 > 0).all() and (S1 > 0).all() and (Sf > 0).all(), \
        "BN scale fold requires positive scale"

    # y_true = S * relu(acc + C/S); S folded into the consumer weights.
    W0 = np.asarray(params["W0"], np.float32)
    W1 = S0[:, None] * np.asarray(params["W1"], np.float32)
    Wf1 = S1[:, None] * np.asarray(params["Wf1"], np.float32)
    Wf2 = Sf[:, None] * np.asarray(params["Wf2"], np.float32)
    c0row = (C0 / S0).astype(np.float32)[None, :]
    c1row = (C1 / S1).astype(np.float32)[None, :]
    cfrow = (Cf / Sf).astype(np.float32)[None, :]

    HID = W0.shape[1]
    HHID = Wf1.shape[1]
    NCLASS = Wf2.shape[1]

    ident = np.eye(128, dtype=np.float32)

    NSH, NB = st.NSH, st.NB
    xv = np.asarray(x, np.float32)

    in_maps = []
    for c in range(ncores):
        idx_cols, btg = st.core_streams(c, dinv, batch)
        xpad = np.zeros((NB * P, INDIM), np.float32)
        xpad[:NSH] = xv[c * NSH:(c + 1) * NSH]
        dinvb = np.zeros((128, NB), np.float32)
        sh_dinv = dinv[c * NSH:(c + 1) * NSH]
        for b in range(NB):
            nb = P if b < NB - 1 else st.LB
            dinvb[:nb, b] = sh_dinv[b * P:b * P + nb]
        in_maps.append(dict(
            xT=np.ascontiguousarray(xpad.T),
            idxs=idx_cols, btiles=btg, dinvb=dinvb,
            w0=W0, w1=W1, wf1=Wf1, wf2=Wf2,
            c0row=c0row, c1row=c1row, cfrow=cfrow,
            ones1=np.ones((1, 128), np.float32),
            bf2b=np.tile(np.asarray(params["bf2"], np.float32)[None, :], (128, 1)),
            invcnt=st.inv_cnt[:, None].copy(),
            ident=ident,
        ))
    dims = dict(INDIM=INDIM, HID=HID, HHID=HHID, NCLASS=NCLASS)
    return st, in_maps, dims


# ------------------------------------------------------------- bass program --

def build_nc(st, dims, ncores, reps=1, fake_coll=False):
    from concourse import bass, mybir, bacc, tile

    INDIM, HID, HHID, NCLASS = dims["INDIM"], dims["HID"], dims["HHID"], dims["NCLASS"]
    N, G, NB, NSH, WS, LB = st.N, st.G, st.NB, st.NSH, st.WS, st.LB
    f32 = mybir.dt.float32
    bf16 = mybir.dt.bfloat16
    Alu = mybir.AluOpType
    Act = mybir.ActivationFunctionType

    nc = bacc.Bacc("TRN2", target_bir_lowering=False, debug=False,
                   enable_asserts=True, num_devices=ncores,
                   num_swdge_queues=NQUEUES)

    I = {}
    def inp(name, shape, dt=f32):
        I[name] = nc.dram_tensor(name, shape, dt, kind="ExternalInput")
        return I[name]

    inp("xT", [INDIM, NB * P])
    inp("idxs", [128, st.SID], mybir.dt.int16)
    f8 = mybir.dt.float8e4
    inp("btiles", [st.NG8 * 128, 8 * 128], f8)
    inp("dinvb", [128, NB])
    inp("w0", [INDIM, HID]); inp("w1", [HID, HID])
    inp("wf1", [HID, HHID]); inp("wf2", [HHID, NCLASS])
    inp("c0row", [1, HID]); inp("c1row", [1, HID]); inp("cfrow", [1, HHID])
    inp("ones1", [1, 128])
    inp("bf2b", [128, NCLASS])
    inp("invcnt", [128, 1])
    inp("ident", [128, 128])
    out_d = nc.dram_tensor("out", [G, NCLASS], f32, kind="ExternalOutput")

    qctr = [0]
    def next_q():
        q = qctr[0] % NQUEUES
        qctr[0] += 1
        return q

    with tile.TileContext(nc) as tc:
        import contextlib
        with contextlib.ExitStack() as ctx:
            const = ctx.enter_context(tc.tile_pool(name="const", bufs=1))
            stream = ctx.enter_context(tc.tile_pool(name="stream", bufs=1))
            xio = ctx.enter_context(tc.tile_pool(name="xio", bufs=3))
            xts = ctx.enter_context(tc.tile_pool(name="xts", bufs=3))
            htab = ctx.enter_context(tc.tile_pool(name="htab", bufs=2))
            ypool = ctx.enter_context(tc.tile_pool(name="ypool", bufs=NB))
            gpool = ctx.enter_context(tc.tile_pool(name="gpool", bufs=GBUFS))
            btpool = ctx.enter_context(tc.tile_pool(name="btpool", bufs=BTBUFS))
            tmp = ctx.enter_context(tc.tile_pool(name="tmp", bufs=6))
            ptrans = ctx.enter_context(tc.tile_pool(name="ptrans", bufs=1, space="PSUM"))
            phw = ctx.enter_context(tc.tile_pool(name="phw", bufs=2, space="PSUM"))
            pacc = ctx.enter_context(tc.tile_pool(name="pacc", bufs=PACCB, space="PSUM"))
            dram = ctx.enter_context(tc.tile_pool(name="dram", bufs=1, space="DRAM"))

            # ---- constants into SBUF
            C = {}
            for nm in ["w0", "w1", "wf1", "wf2", "c0row", "c1row", "cfrow",
                       "ones1", "bf2b", "invcnt", "ident"]:
                shape = list(I[nm].shape)
                tile_ = const.tile(shape, f32, tag=nm)
                nc.sync.dma_start(out=tile_[:], in_=I[nm][:])
                C[nm] = tile_
            idx_t = stream.tile([128, st.SID], mybir.dt.int16, tag="idx")
            nc.sync.dma_start(out=idx_t[:], in_=I["idxs"][:])
            dinv_t = stream.tile([128, NB], f32, tag="dnv")
            nc.sync.dma_start(out=dinv_t[:], in_=I["dinvb"][:])

            shspace = "Shared" if (ncores > 4 and reps == 1) else "Local"
            H2 = st.H2
            bnc00 = dram.tile([H2, 2 * HID], bf16, tag="bnc00")
            bnc01 = dram.tile([H2, 2 * HID], bf16, tag="bnc01")
            bnc10 = dram.tile([H2, 2 * HID], bf16, tag="bnc10")
            bnc11 = dram.tile([H2, 2 * HID], bf16, tag="bnc11")
            tab00 = dram.tile([N // 2, 2 * HID], bf16, tag="tab00", addr_space=shspace)
            tab01 = dram.tile([N // 2, 2 * HID], bf16, tag="tab01", addr_space=shspace)
            tab10 = dram.tile([N // 2, 2 * HID], bf16, tag="tab10", addr_space=shspace)
            tab11 = dram.tile([N // 2, 2 * HID], bf16, tag="tab11", addr_space=shspace)
            bounce = [[bnc00, bnc01], [bnc10, bnc11]]
            table = [[tab00, tab01], [tab10, tab11]]
            ar_in = dram.tile([G, HID], f32, tag="arin")
            ar_out = dram.tile([G, HID], f32, tag="arout", addr_space=shspace)

            # B-tile group fetch machinery (sequential stream, groups of 8)
            class BTStream:
                def __init__(self):
                    self.pos = 0
                    self.cur = None
                def reset(self, pos):
                    self.pos = pos
                    self.cur = None
                def next(self):
                    g, slot = divmod(self.pos, 8)
                    if slot == 0 or self.cur is None:
                        self.cur = btpool.tile([128, 8 * 128], f8, tag="bt")
                        nc.sync.dma_start(
                            out=self.cur[:],
                            in_=I["btiles"][g * 128:(g + 1) * 128, :])
                    self.pos += 1
                    return self.cur[:, slot * 128:(slot + 1) * 128]
            bts = BTStream()

            for _rep in range(reps):
              # ---- phase A: table0 rows = dinv * (x @ W0) -> AllGather
              htb = htab.tile([128, NB, 2 * HID], bf16, tag="htb")
              nc.vector.memset(htb[:], 0)
              xtt = None
              for b in range(NB):
                  if b % 8 == 0:
                      nbk = min(8, NB - b)
                      xtt = xio.tile([128, 8 * 128], f32, tag="xt")
                      nc.sync.dma_start(out=xtt[:, :nbk * 128],
                                        in_=I["xT"][:, b * P:(b + nbk) * P])
                  hp = phw.tile([128, HID], f32, tag="hp")
                  nc.tensor.matmul(hp[:], lhsT=xtt[:, (b % 8) * 128:(b % 8 + 1) * 128],
                                   rhs=C["w0"][:], start=True, stop=True)
                  nc.scalar.activation(out=htb[:, b, :HID], in_=hp[:], func=Act.Copy,
                                       scale=dinv_t[:, b:b + 1])
              def bounce_write(l, srctile, b, nb):
                  eng = nc.sync if b % 2 == 0 else nc.scalar
                  g0, g1 = b * P, b * P + nb
                  if g1 <= H2:
                      eng.dma_start(out=bounce[l][0][g0:g1, :],
                                    in_=srctile[:nb, b, :])
                  elif g0 >= H2:
                      eng.dma_start(out=bounce[l][1][g0 - H2:g1 - H2, :],
                                    in_=srctile[:nb, b, :])
                  else:
                      k = H2 - g0
                      eng.dma_start(out=bounce[l][0][g0:H2, :],
                                    in_=srctile[:k, b, :])
                      eng.dma_start(out=bounce[l][1][0:nb - k, :],
                                    in_=srctile[k:nb, b, :])

              for b in range(NB):
                  nb = P if b < NB - 1 else LB
                  bounce_write(0, htb, b, nb)
              h16_big = htb

              for h in range(2):
                  if fake_coll:
                      nc.sync.dma_start(out=table[0][h][0:H2, :], in_=bounce[0][h][:])
                  else:
                      nc.gpsimd.collective_compute(
                          "AllGather", Alu.bypass,
                          replica_groups=[list(range(ncores))],
                          ins=[bounce[0][h].opt()], outs=[table[0][h].opt()],
                      )

              # ---- GCN layers
              y_tiles = None
              for l in range(2):
                  crow = C["c0row"] if l == 0 else C["c1row"]
                  ydt = f32 if l == 0 else bf16
                  bts.reset(0)
                  new_tiles = []
                  for sgi, sg in enumerate(st.sgs):
                      qctr[0] += 1
                      gt = {}
                      for w in range(NWIN):
                          cols = st.gcols[(sgi, w)]
                          if cols == 0:
                              continue
                          gbf = gpool.tile([128, st.GMAX, 2 * HID], bf16, tag="g")
                          gt[w] = gbf
                          hw_, wl = divmod(w, NWIN // 2)
                          nc.gpsimd.dma_gather(
                              out_ap=gbf[:, :cols, :],
                              in_ap=table[l][hw_][wl * WS:min((wl + 1) * WS, N // 2), :],
                              idxs_ap=idx_t[:, st.icol[(sgi, w)]:st.icol[(sgi, w)] + cols * 8],
                              num_idxs=cols * P,
                              num_idxs_reg=cols * P,
                              elem_size=2 * HID,
                              queue_num=next_q(),
                              single_packet=SINGLE_PACKET,
                          )
                      for b in sg:
                          nchunks = int(st.cap[b].sum()) // P
                          total = nchunks + 2  # + self-diag + bias row
                          acc = pacc.tile([128, HID], f32, tag="acc")
                          done = 0
                          for w in range(NWIN):
                              kk = int(st.cap[b, w]) // P
                              for j in range(kk):
                                  Bt = bts.next()
                                  nc.tensor.matmul(
                                      acc[:], lhsT=Bt,
                                      rhs=gt[w][:, st.coloff[(b, w)] + j, 0:HID],
                                      start=(done == 0), stop=False)
                                  done += 1
                          Dt = bts.next()  # self-loop diag(dinv_dst)
                          nc.tensor.matmul(acc[:], lhsT=Dt,
                                           rhs=h16_big[:, b, 0:HID],
                                           start=(done == 0), stop=False)
                          nc.tensor.matmul(acc[:], lhsT=C["ones1"][:1, :],
                                           rhs=crow[:1, :], start=False, stop=True)
                          yb = ypool.tile([128, HID], ydt, tag="y")
                          nc.scalar.activation(out=yb[:], in_=acc[:], func=Act.Relu)
                          new_tiles.append(yb)
                  y_tiles = new_tiles

                  if l == 0:
                      # table1 rows = dinv * (y0 @ W1') -> AllGather
                      htb1 = htab.tile([128, NB, 2 * HID], bf16, tag="htb")
                      nc.vector.memset(htb1[:], 0)
                      for b in range(NB):
                          yb = y_tiles[b]
                          pt = ptrans.tile([128, 128], f32, tag="pt")
                          nc.tensor.transpose(pt[:HID, :], yb[:], C["ident"][:])
                          yTs = xts.tile([128, 128], f32, tag="xT")
                          nc.scalar.activation(out=yTs[:HID, :], in_=pt[:HID, :],
                                               func=Act.Copy)
                          hp = phw.tile([128, HID], f32, tag="hp")
                          nc.tensor.matmul(hp[:], lhsT=yTs[:HID, :], rhs=C["w1"][:],
                                           start=True, stop=True)
                          nc.scalar.activation(out=htb1[:, b, :HID], in_=hp[:],
                                               func=Act.Copy,
                                               scale=dinv_t[:, b:b + 1])
                      for b in range(NB):
                          nb = P if b < NB - 1 else LB
                          bounce_write(1, htb1, b, nb)
                      h16_big = htb1
                      for h in range(2):
                          if fake_coll:
                              nc.sync.dma_start(out=table[1][h][0:H2, :],
                                                in_=bounce[1][h][:])
                          else:
                              nc.gpsimd.collective_compute(
                                  "AllGather", Alu.bypass,
                                  replica_groups=[list(range(ncores))],
                                  ins=[bounce[1][h].opt()], outs=[table[1][h].opt()],
                              )

              # ---- mean pool (partial per core, AllReduce) + head
              pp = pacc.tile([128, HID], f32, tag="acc")
              for b in range(NB):
                  Bp = bts.next()
                  nc.tensor.matmul(pp[:G, :], lhsT=Bp[:, :G], rhs=y_tiles[b][:],
                                   start=(b == 0), stop=(b == NB - 1))
              pooled = tmp.tile([128, HID], f32, tag="pl")
              nc.scalar.activation(out=pooled[:G, :], in_=pp[:G, :], func=Act.Copy)
              nc.sync.dma_start(out=ar_in[:], in_=pooled[:G, :])
              if fake_coll:
                  nc.sync.dma_start(out=ar_out[:], in_=ar_in[:])
              else:
                  nc.gpsimd.collective_compute(
                      "AllReduce", Alu.add,
                      replica_groups=[list(range(ncores))],
                      ins=[ar_in.opt()], outs=[ar_out.opt()],
                  )
              pooled2 = tmp.tile([128, HID], f32, tag="pl2")
              nc.sync.dma_start(out=pooled2[:G, :], in_=ar_out[:])
              nc.vector.tensor_scalar(out=pooled2[:G, :], in0=pooled2[:G, :],
                                      scalar1=C["invcnt"][:G, :], scalar2=None,
                                      op0=Alu.mult)

              # z = relu((pooled @ Wf1') + Cf')
              pt = ptrans.tile([128, 128], f32, tag="pt")
              nc.tensor.transpose(pt[:HID, :G], pooled2[:G, :], C["ident"][:])
              pTs = xts.tile([128, 128], f32, tag="xT")
              nc.scalar.activation(out=pTs[:HID, :G], in_=pt[:HID, :G], func=Act.Copy)
              zp = phw.tile([128, HHID], f32, tag="hp")
              nc.tensor.matmul(zp[:G, :], lhsT=pTs[:HID, :G], rhs=C["wf1"][:],
                               start=True, stop=False)
              nc.tensor.matmul(zp[:G, :], lhsT=C["ones1"][:1, :G],
                               rhs=C["cfrow"][:1, :], start=False, stop=True)
              z = tmp.tile([128, HHID], f32, tag="z")
              nc.scalar.activation(out=z[:G, :], in_=zp[:G, :], func=Act.Relu)

              # logits = z @ Wf2' + bf2; out = log_softmax(logits)
              pt2 = ptrans.tile([128, 128], f32, tag="pt")
              nc.tensor.transpose(pt2[:HHID, :G], z[:G, :], C["ident"][:])
              zTs = xts.tile([128, 128], f32, tag="xT")
              nc.scalar.activation(out=zTs[:HHID, :G], in_=pt2[:HHID, :G], func=Act.Copy)
              lp = phw.tile([128, NCLASS], f32, tag="hp")
              nc.tensor.matmul(lp[:G, :], lhsT=zTs[:HHID, :G], rhs=C["wf2"][:],
                               start=True, stop=True)
              lg = tmp.tile([128, NCLASS], f32, tag="lg")
              nc.vector.tensor_tensor(out=lg[:G, :], in0=lp[:G, :], in1=C["bf2b"][:G, :], op=Alu.add)
              mx = tmp.tile([128, 1], f32, tag="mx")
              nc.vector.reduce_max(mx[:G, :], lg[:G, :], axis=mybir.AxisListType.X)
              nc.vector.tensor_scalar(out=lg[:G, :], in0=lg[:G, :], scalar1=mx[:G, :],
                                      scalar2=None, op0=Alu.subtract)
              ex = tmp.tile([128, NCLASS], f32, tag="ex")
              nc.scalar.activation(out=ex[:G, :], in_=lg[:G, :], func=Act.Exp)
              sm = tmp.tile([128, 1], f32, tag="sm")
              nc.vector.reduce_sum(sm[:G, :], ex[:G, :], axis=mybir.AxisListType.X)
              lsm = tmp.tile([128, 1], f32, tag="ls")
              nc.scalar.activation(out=lsm[:G, :], in_=sm[:G, :], func=Act.Ln)
              nc.vector.tensor_scalar(out=lg[:G, :], in0=lg[:G, :], scalar1=lsm[:G, :],
                                      scalar2=None, op0=Alu.subtract)
              nc.sync.dma_start(out=out_d[:], in_=lg[:G, :])

    nc.compile()
    return nc


# ------------------------------------------------------------ PJRT runner --

class SpmdRunner:
    """Run the compiled 8-core Bass module via PJRT (axon), mirroring
    concourse.bass2jax.run_bass_via_pjrt but keeping the jitted callable."""

    def __init__(self, nc, n_cores):
        import jax
        from jax.sharding import Mesh, PartitionSpec
        from jax.experimental.shard_map import shard_map
        from concourse import bass2jax, mybir as _mb
        from concourse.bass2jax import _bass_exec_p, install_neuronx_cc_hook
        install_neuronx_cc_hook()
        self.jax = jax
        self.nc = nc
        self.n_cores = n_cores
        partition_name = nc.partition_id_tensor.name if nc.partition_id_tensor else None
        in_names, out_names, out_avals, zero_outs = [], [], [], []
        for alloc in nc.m.functions[0].allocations:
            if not isinstance(alloc, _mb.MemoryLocationSet):
                continue
            name = alloc.memorylocations[0].name
            if alloc.kind == "ExternalInput":
                if name != partition_name:
                    in_names.append(name)
            elif alloc.kind == "ExternalOutput":
                shape = tuple(alloc.tensor_shape)
                dtype = _mb.dt.np(alloc.dtype)
                out_names.append(name)
                out_avals.append(jax.core.ShapedArray(shape, dtype))
                zero_outs.append(np.zeros(shape, dtype))
        self.in_names, self.out_names = in_names, out_names
        self.out_avals, self.zero_outs = out_avals, zero_outs
        n_params, n_outs = len(in_names), len(out_avals)
        self.n_params = n_params
        all_in_names = in_names + out_names + ([partition_name] if partition_name else [])

        def _body(*args):
            operands = list(args)
            if partition_name is not None:
                operands.append(bass2jax.partition_id_tensor())
            return tuple(_bass_exec_p.bind(
                *operands, out_avals=tuple(out_avals), in_names=tuple(all_in_names),
                out_names=tuple(out_names), lowering_input_output_aliases=(),
                sim_require_finite=True, sim_require_nnan=True, nc=nc))

        devices = jax.devices()[:n_cores]
        assert len(devices) == n_cores
        mesh = Mesh(np.asarray(devices), ("core",))
        self._sharding = jax.sharding.NamedSharding(mesh, PartitionSpec("core"))
        in_specs = (PartitionSpec("core"),) * (n_params + n_outs)
        out_specs = (PartitionSpec("core"),) * len(out_names)
        self._fn = jax.jit(
            shard_map(_body, mesh=mesh, in_specs=in_specs,
                      out_specs=out_specs, check_rep=False),
            keep_unused=True)

    def prepare(self, in_maps):
        per_core = [[np.asarray(m[name]) for name in self.in_names] for m in in_maps]
        concat_in = [np.concatenate([per_core[c][i] for c in range(self.n_cores)], axis=0)
                     for i in range(self.n_params)]
        concat_zeros = [np.zeros((self.n_cores * z.shape[0], *z.shape[1:]), z.dtype)
                        for z in self.zero_outs]
        return concat_in + concat_zeros

    def run(self, in_maps):
        out_arrs = self._fn(*self.prepare(in_maps))
        self.jax.block_until_ready(out_arrs)
        return self._split(out_arrs)

    def _split(self, out_arrs):
        return [{name: np.asarray(out_arrs[i]).reshape(self.n_cores, *self.out_avals[i].shape)[c]
                 for i, name in enumerate(self.out_names)}
                for c in range(self.n_cores)]

    def time(self, in_maps, iters=8):
        import time as _t
        args = self.prepare(in_maps)
        dargs = [self.jax.device_put(a, self._sharding) for a in args]
        out = self._fn(*dargs)
        self.jax.block_until_ready(out)
        results = self._split(out)
        times = []
        for _ in range(iters):
            t0 = _t.perf_counter()
            o = self._fn(*dargs)
            self.jax.block_until_ready(o)
            times.append(_t.perf_counter() - t0)
        return results, times


# ------------------------------------------------------------------- driver --

_CACHE = {}


def _get_runner(st, dims, ncores):
    nc = build_nc(st, dims, ncores)
    return SpmdRunner(nc, ncores)


def kernel(**inputs):
    x = np.asarray(inputs["x"], np.float32)
    edge_index = np.asarray(inputs["edge_index"])
    batch = np.asarray(inputs["batch"])
    edge_attr = np.asarray(inputs["edge_attr"], np.float32)
    G = 128
    params = {k: np.asarray(v) for k, v in inputs.items()
              if k not in ("x", "edge_index", "batch", "edge_attr", "pos")}
    params["cnt_G"] = G
    ncores = 8

    st, in_maps, dims = _host_prep(x, edge_index, batch, edge_attr, params, ncores)

    key = ("k2", x.shape, edge_index.shape, st.SID, st.CT, st.GMAX,
           tuple(tuple(s) for s in st.sgs))
    if key not in _CACHE:
        _CACHE[key] = _get_runner(st, dims, ncores)
    runner = _CACHE[key]
    _LAST.update(st=st, dims=dims, ncores=ncores, in_maps=in_maps, runner=runner)
    results = runner.run(in_maps)
    return results[0]["out"]


_LAST = {}


def estimate_exec_ns(reps=16, iters=10):
    """Per-execution device time via wall-clock delta between a 1-rep NEFF and
    an in-NEFF `reps`-times-repeated body (cancels the axon dispatch floor).
    Median-based: the axon tunnel has heavy-tailed per-call jitter."""
    import time as _t
    import jax
    st, dims, ncores = _LAST["st"], _LAST["dims"], _LAST["ncores"]
    in_maps, r1 = _LAST["in_maps"], _LAST["runner"]
    rR = SpmdRunner(build_nc(st, dims, ncores, reps=reps), ncores)
    a1 = [jax.device_put(a, r1._sharding) for a in r1.prepare(in_maps)]
    aR = [jax.device_put(a, rR._sharding) for a in rR.prepare(in_maps)]
    jax.block_until_ready(r1._fn(*a1)); jax.block_until_ready(rR._fn(*aR))
    t1s, tRs = [], []
    for _ in range(iters):
        t0 = _t.perf_counter(); jax.block_until_ready(r1._fn(*a1)); t1s.append(_t.perf_counter() - t0)
        t0 = _t.perf_counter(); jax.block_until_ready(rR._fn(*aR)); tRs.append(_t.perf_counter() - t0)
    t1s, tRs = sorted(t1s), sorted(tRs)
    per = (tRs[len(tRs) // 2] - t1s[len(t1s) // 2]) / (reps - 1)
    return per * 1e9
